# revision 41
# baseline (speedup 1.0000x reference)
"""Trainium2 8-core kernel for nn_EnhancedTransformerBlock (v2).

SPMD: identical program on all 8 cores, only in_maps data differs.
  - Sequence-sharded everywhere except attention: core c owns tokens
    [256c, 256c+256), activations in T-layout [feature, token].
  - Attention head-sharded (2 of 16 heads per core, full sequence).
    AllGather of ln(x) (bf16) before QKV; AllToAll of per-head attention
    outputs back to sequence sharding. A dummy tiny collective is issued
    first to absorb the NRT bootstrap barrier during input DMA/LN.
  - All GEMMs bf16 (weights pre-transposed/packed host-side), fp32 PSUM.
  - Softmax: temperature (1/0.1) and 1/sqrt(hd) folded into Wq; unshifted
    exp; denominator via ones-column appended to V; causal masking via
    triangle-mask multiply on diagonal blocks. The entropy gate (ent) is
    folded into the V GEMM as a 137th output column.
  - Only exp-set scalar activations are used (Exp/Abs/Identity/Square/
    Relu/Copy all live in the exp_and_others ACT table set) -> exactly one
    ACT_TABLE_LOAD. All rsqrt row math uses a Quake-style bit hack + two
    Newton steps on the vector engine.
  - FFN: mean/var of h computed directly from x1 via host-precomputed
    folds (row-sum vector for the mean; Gram matrix G = W1^T W1 for the
    sum of squares), so the spline scale S = rsqrt(var+eps)/(1+sqrt(FF))
    is ready before ff1 finishes. (1+norm) == 1+sqrt(FF) to ~1e-6 rel.
  - ep gate path contracted over D instead of FF via Wc = ep1_w @ ff1_w
    (host precompute), so it runs in parallel with ff1.
  - Spline activation g(u) approximated by a 4-term basis [1, u, u^2,
    |u|] LSQ-fit on |u|<=0.15 (|u| < 0.08 in practice); the quadratic
    part is computed with one scalar-engine Square via completing the
    square. ff2 is interleaved per 8-chunk group with the spline.
"""

import hashlib
import numpy as np

from concourse import bacc, tile, mybir
from concourse import bass_utils

dt = mybir.dt
BF = dt.bfloat16
F32 = dt.float32
I32 = dt.int32
NPBF = dt.np(BF)
Alu = mybir.AluOpType
Act = mybir.ActivationFunctionType

NCORES = 8
S = 2048
D = 1024
H = 16
HD = 64
FF = 4096
D16 = 256
TOK = S // NCORES            # 256 tokens per core
HPC = H // NCORES            # 2 heads per core
EPS = 1e-6
UDOM = 0.15                  # spline fit domain |u| <= UDOM
VW = 137                     # augmented V width: 2*68 + ent column
QK_C = 0x5F3759E0            # quake magic + 1 (for the xor/add form)

_prog_cache = {}


# ----------------------------------------------------------------------------
# Host-side: spline fit
# ----------------------------------------------------------------------------

def _g_exact(u, knots, spl_w):
    d = np.abs(u[:, None] - knots[None, :])
    d = d / (d.max(-1, keepdims=True) + EPS)
    a = -5.0 * d
    a = a - a.max(-1, keepdims=True)
    e = np.exp(a)
    p = e / e.sum(-1, keepdims=True)
    return (p * spl_w).sum(-1)


def _fit_spline(knots, spl_w):
    """LSQ fit of g(u) on [-UDOM, UDOM] with basis [1, u, u^2, |u|].
    Returns dict with the square-trick constants."""
    k = np.asarray(knots, np.float64)
    w = np.asarray(spl_w, np.float64)
    u = np.linspace(-UDOM, UDOM, 20001)
    B = np.stack([np.ones_like(u), u, u * u, np.abs(u)], -1)
    y = _g_exact(u, k, w)
    c, *_ = np.linalg.lstsq(B, y, rcond=None)
    err = float(np.abs(B @ c - y).max())
    c0, c1, c2, c3 = (float(v) for v in c)
    s2 = 1.0 if c2 >= 0 else -1.0
    a = max(np.sqrt(abs(c2)), 1e-3)
    dq = c1 / (2.0 * s2 * a)
    c0p = c0 - s2 * dq * dq + s2 * a * a * 0.0
    # residual error from the a-floor when |c2| tiny:
    # (a^2 - |c2|) * u^2 <= (1e-6)*UDOM^2 -- negligible.
    return {"a": float(a), "d": float(dq), "s2": s2, "c0p": float(c0p),
            "c3": c3, "fit_err": err}


# ----------------------------------------------------------------------------
# Host-side: weight packing
# ----------------------------------------------------------------------------

def _pack_lhsT(w_t, n_of, n_kc, kc_major=False):
    """w_t: [K_total, M_total] ([in, out]) -> [128, n_of*n_kc*128].
    of-major tile order by default; kc-major if requested."""
    K_total, M_total = w_t.shape
    assert K_total == n_kc * 128 and M_total == n_of * 128
    out = np.empty((128, n_of * n_kc * 128), np.float32)
    for of in range(n_of):
        for kc in range(n_kc):
            idx = (kc * n_of + of) if kc_major else (of * n_kc + kc)
            out[:, idx * 128:(idx + 1) * 128] = \
                w_t[kc * 128:(kc + 1) * 128, of * 128:(of + 1) * 128]
    return np.ascontiguousarray(out)


def _col_pack(vec, n_chunks):
    return np.ascontiguousarray(
        np.asarray(vec, np.float32).reshape(n_chunks, 128).T)


def _make_tri_masks():
    out = np.zeros((128, 4 * 512), np.float32)
    for j in range(4):
        kk = np.arange(128)[:, None] + 128 * j
        q = np.arange(512)[None, :]
        out[:, 512 * j:512 * (j + 1)] = (kk <= q).astype(np.float32)
    return out


def _prepare_inputs(inputs):
    f = lambda k: np.asarray(inputs[k], np.float32)
    x = f("x").reshape(S, D)
    qkv_w, qkv_b = f("qkv_w"), f("qkv_b")
    out_w, out_b = f("out_w") * 0.1, f("out_b") * 0.1
    ff1_w, ff1_b = f("ff1_w"), f("ff1_b")
    ff2_w, ff2_b = f("ff2_w"), f("ff2_b")
    ep1_w, ep1_b = f("ep1_w"), f("ep1_b")
    ep2_w, ep2_b = f("ep2_w"), f("ep2_b")
    ent_w, ent_b = f("ent_w"), f("ent_b")

    temp = (1.0 / np.sqrt(np.float32(HD))) / 0.1   # 1.25
    wq = qkv_w[0:D] * temp
    wk = qkv_w[D:2 * D]
    wv = qkv_w[2 * D:3 * D]
    bq = qkv_b[0:D] * temp
    bk = qkv_b[D:2 * D]
    bv = qkv_b[2 * D:3 * D]

    spl = _fit_spline(f("knots"), f("spl_w"))

    # ep-path fold: h @ ep1_w.T = x1 @ (ep1_w @ ff1_w).T + ep1_w @ ff1_b
    wc = (ep1_w.astype(np.float64) @ ff1_w.astype(np.float64)).astype(np.float32)
    bc = ep1_b + ep1_w @ ff1_b
    # mean of h fold
    wsum = ff1_w.sum(0) / FF                        # [D]
    bsum = float(ff1_b.sum()) / FF
    # sum-of-squares fold: G = W1^T W1, linear term, const term
    G = (ff1_w.T.astype(np.float64) @ ff1_w.astype(np.float64)).astype(np.float32)
    c_lin = 2.0 * (ff1_b @ ff1_w)                   # [D]
    btb = float(ff1_b @ ff1_b)

    # consolidated f32 constants: one DMA instead of ~20
    cpack = np.concatenate([
        np.ones((128, 1), np.float32),      # ones32      0:1
        _col_pack(out_b, 8),                # b_out       1:9
        _col_pack(ff1_b, 32),               # b_ff1       9:41
        _col_pack(ff2_b, 8),                # b_ff2      41:49
        _col_pack(bc, 2),                   # b_epc      49:51
        _col_pack(c_lin, 8),                # c_lin      51:59
        _col_pack(f("ln_attn_w"), 8),       # lnw        59:67
        _col_pack(f("ln_attn_b"), 8),       # lnb        67:75
        _col_pack(f("norm1_w"), 8),         # n1w        75:83
        _col_pack(f("norm1_b"), 8),         # n1b        83:91
        _col_pack(f("norm2_w"), 8),         # n2w        91:99
        _col_pack(f("norm2_b"), 8),         # n2b        99:107
        _col_pack(f("ep_ln_w"), 2),         # eplw      107:109
        _col_pack(f("ep_ln_b"), 2),         # eplb      109:111
    ], 1)
    bpack = np.concatenate([
        np.ones((128, 1), np.float32),      # onesb       0:1
        _col_pack(wsum, 8),                 # wsum        1:9
        np.ascontiguousarray(ep2_w.reshape(2, 128).T),  # wep2 9:11
    ], 1).astype(NPBF)
    shared = {
        "tri": _make_tri_masks().astype(NPBF),
        "cpack": cpack,
        "bpack": bpack,
        "wff1": _pack_lhsT(ff1_w.T, 32, 8).astype(NPBF),
        "wff2": _pack_lhsT(ff2_w.T, 8, 32, kc_major=True).astype(NPBF),
        "wepc": _pack_lhsT(wc.T, 2, 8).astype(NPBF),
        "wgram": _pack_lhsT(G, 8, 8).astype(NPBF),
        "wout": _pack_lhsT(out_w.T, 8, 8).astype(NPBF),
    }

    scalars = {
        "ent_b": float(ent_b.reshape(-1)[0]),
        "ep2_b": float(ep2_b.reshape(-1)[0]),
        "bsum": bsum,
        "btb": btb,
        "spl": spl,
    }

    in_maps = []
    for c in range(NCORES):
        m = dict(shared)
        xc = x[c * TOK:(c + 1) * TOK]                        # [256, D]
        xT = np.ascontiguousarray(xc.T)                      # [D, 256]
        m["xT"] = np.ascontiguousarray(
            xT.reshape(8, 128, TOK).transpose(1, 0, 2).reshape(128, 8 * TOK))
        h0 = c * HPC
        wq_c = wq[h0 * HD:(h0 + HPC) * HD]                   # [128, D]
        wk_c = wk[h0 * HD:(h0 + HPC) * HD]
        wqk_t = np.concatenate([wq_c, wk_c], 0).T            # [D, 256]
        m["wqk"] = _pack_lhsT(wqk_t, 2, 8).astype(NPBF)
        m["b_qk"] = np.ascontiguousarray(np.stack(
            [bq[h0 * HD:(h0 + HPC) * HD],
             bk[h0 * HD:(h0 + HPC) * HD]], -1).astype(np.float32))
        wv_c = wv[h0 * HD:(h0 + HPC) * HD].T                 # [D, 128]
        wva = np.zeros((D, VW), np.float32)
        bva = np.zeros((1, VW), np.float32)
        for lh in range(HPC):
            wva[:, 68 * lh:68 * lh + 64] = wv_c[:, 64 * lh:64 * lh + 64]
            bva[0, 68 * lh:68 * lh + 64] = \
                bv[(h0 + lh) * HD:(h0 + lh + 1) * HD]
        wva[:, 136] = ent_w[0]                               # ent gate column
        m["wv"] = np.ascontiguousarray(
            wva.reshape(8, 128, VW).transpose(1, 0, 2).reshape(128, 8 * VW)
        ).astype(NPBF)
        m["bvb"] = np.ascontiguousarray(np.tile(bva, (128, 1)))
        in_maps.append(m)

    return in_maps, scalars


# ----------------------------------------------------------------------------
# Device program helpers
# ----------------------------------------------------------------------------

def _quake_rsqrt(nc, out, v, t_i, y_f, t2_f, scale=1.0):
    """out = scale / sqrt(v) elementwise on f32 row APs, vector engine only.
    t_i (int32-viewable f32 tile), y_f, t2_f are scratch APs, same shape."""
    v_ = nc.vector
    # y0 bits = C - (v_bits >> 1)  ==  ((v>>1) ^ ~0) + (C+1)
    v_.tensor_scalar(t_i.bitcast(I32), v.bitcast(I32), 1, -1,
                     Alu.arith_shift_right, Alu.bitwise_xor)
    v_.tensor_scalar(y_f.bitcast(I32), t_i.bitcast(I32), QK_C, None, Alu.add)
    # newton 1: y = y*(1.5 - 0.5*v*y*y)
    v_.tensor_tensor(t_i, y_f, y_f, Alu.mult)
    v_.tensor_tensor(t_i, t_i, v, Alu.mult)
    v_.tensor_scalar(t2_f, t_i, -0.5, 1.5, Alu.mult, Alu.add)
    v_.tensor_tensor(y_f, t2_f, y_f, Alu.mult)
    # newton 2 (scaled): out = scale * y*(1.5 - 0.5*v*y*y)
    v_.tensor_tensor(t_i, y_f, y_f, Alu.mult)
    v_.tensor_tensor(t_i, t_i, v, Alu.mult)
    v_.tensor_scalar(t2_f, t_i, -0.5 * scale, 1.5 * scale, Alu.mult, Alu.add)
    v_.tensor_tensor(out, t2_f, y_f, Alu.mult)


def _build_program(sc):
    nc = bacc.Bacc("TRN2", target_bir_lowering=False, debug=False,
                   num_devices=NCORES)

    def din(name, shape, dtype):
        return nc.dram_tensor(name, list(shape), dtype, kind="ExternalInput")

    tin = {
        "xT": din("xT", (128, 8 * TOK), F32),
        "wqk": din("wqk", (128, 2048), BF),
        "wv": din("wv", (128, 8 * VW), BF),
        "wout": din("wout", (128, 8192), BF),
        "wff1": din("wff1", (128, 32768), BF),
        "wff2": din("wff2", (128, 32768), BF),
        "wepc": din("wepc", (128, 2048), BF),
        "wgram": din("wgram", (128, 8192), BF),
        "tri": din("tri", (128, 2048), BF),
        "cpack": din("cpack", (128, 111), F32),
        "bpack": din("bpack", (128, 11), BF),
        "b_qk": din("b_qk", (128, 2), F32),
        "bvb": din("bvb", (128, VW), F32),
    }
    t_out = nc.dram_tensor("out", [128, 8 * TOK], F32, kind="ExternalOutput")
    import os
    dbg = {}
    if os.environ.get("KDEBUG", "0") == "1":
        for nm, shape in (("d_xall", (128, 16384)), ("d_qkT", (128, 4096)),
                          ("d_vaug", (128, 16 * VW)), ("d_es", (128, 16)),
                          ("d_aosc", (128, 2048)), ("d_aofull", (128, 8 * TOK)),
                          ("d_x1f", (128, 8 * TOK)), ("d_hb", (128, 8192)),
                          ("d_actt", (128, 8192)), ("d_rows", (1, 16 * TOK)),
                          ("d_u", (128, 8192)), ("d_r2", (128, 8 * TOK))):
            dbg[nm] = nc.dram_tensor(nm, list(shape), F32, kind="ExternalOutput")
    ag_in = nc.dram_tensor("ag_in", [1024, TOK], BF, kind="Internal")
    ag_out = nc.dram_tensor("ag_out", [8192, TOK], BF, kind="Internal",
                            addr_space="Shared")
    a2a_in = nc.dram_tensor("a2a_in", [1024, TOK], BF, kind="Internal")
    a2a_out = nc.dram_tensor("a2a_out", [1024, TOK], BF, kind="Internal")

    with tile.TileContext(nc) as tc:
        _emit(nc, tc, tin, t_out, ag_in, ag_out, a2a_in, a2a_out, sc, dbg)
    nc.compile()
    return nc


def _emit(nc, tc, tin, t_out, ag_in, ag_out, a2a_in, a2a_out, sc, dbg):
    v = nc.vector
    s = nc.scalar
    g = nc.gpsimd
    te = nc.tensor
    dma = nc.sync.dma_start
    spl = sc["spl"]
    RG = [list(range(NCORES))]

    with tc.tile_pool(name="persist", bufs=1) as P, \
         tc.tile_pool(name="consts", bufs=1) as C, \
         tc.tile_pool(name="rows", bufs=1) as R:

        # persistent tiles
        onesr = P.tile([1, 64], BF, tag="onesr")
        xt = P.tile([128, 8 * TOK], F32, tag="xt")
        x1f = P.tile([128, 8 * TOK], F32, tag="x1f")
        x1b = P.tile([128, 8 * TOK], BF, tag="x1b")

        # constants: two packed DMAs + slice views
        cpk = C.tile([128, 111], F32, tag="cpk")
        bpk = C.tile([128, 11], BF, tag="bpk")
        bqk = C.tile([128, 2], F32, tag="bqk")
        bvb = C.tile([128, VW], F32, tag="bvb")
        dma(out=cpk[:], in_=tin["cpack"].ap())
        dma(out=bpk[:], in_=tin["bpack"].ap())
        dma(out=bqk[:], in_=tin["b_qk"].ap())
        dma(out=bvb[:], in_=tin["bvb"].ap())
        _coff = {"ones32": (0, 1), "b_out": (1, 9), "b_ff1": (9, 41),
                 "b_ff2": (41, 49), "b_epc": (49, 51), "c_lin": (51, 59),
                 "lnw": (59, 67), "lnb": (67, 75), "n1w": (75, 83),
                 "n1b": (83, 91), "n2w": (91, 99), "n2b": (99, 107),
                 "eplw": (107, 109), "eplb": (109, 111)}
        sm = {nm: cpk[:, a:b] for nm, (a, b) in _coff.items()}
        sm["onesb"] = bpk[:, 0:1]
        sm["wsum"] = bpk[:, 1:9]
        sm["wep2"] = bpk[:, 9:11]
        sm["b_qk"] = bqk[:]
        ones32, onesb = sm["ones32"], sm["onesb"]
        cst = C.tile([128, 3], F32, tag="cst")
        v.memset(cst[:, 0:1], -sc["ent_b"])
        v.memset(cst[:, 1:2], -sc["ep2_b"])
        v.memset(cst[:, 2:3], sc["spl"]["d"])

        v.memset(onesr[:], 1.0)

        # pool opens (LIFO close order: XA, WA, MID, HB, WBIG, TMP3)
        TMP3_cm = tc.tile_pool(name="tmp3", bufs=1)
        TMP3 = TMP3_cm.__enter__()
        WBIG = tc.tile_pool(name="wbig_pool", bufs=4)
        WBIGp = WBIG.__enter__()
        HB_cm = tc.tile_pool(name="hb_pool", bufs=1)
        HBp = HB_cm.__enter__()
        MID_cm = tc.tile_pool(name="mid_pool", bufs=1)
        MIDp = MID_cm.__enter__()
        qkT = MIDp.tile([128, 4096], BF, tag="qkT")
        vaug = MIDp.tile([128, 16 * VW], BF, tag="vaug")
        aosc = MIDp.tile([128, 2048], BF, tag="aoshare", name="aosc")
        aofull = MIDp.tile([128, 8 * TOK], BF, tag="aoshare", name="aofull")

        # rows: [1, TOK] f32 rows packed in one tile; index by name
        NROW = 14
        rows = R.tile([1, NROW * TOK], F32, tag="rows")
        _r = {}
        for i, nm in enumerate(("mu1", "s1", "ra", "rb", "rc",
                                "muh", "Sh", "muS", "em",
                                "mue", "se", "m1", "m2", "sc1")):
            _r[nm] = rows[0:1, i * TOK:(i + 1) * TOK]
        rs = lambda nm: _r[nm]

        dma(out=xt[:], in_=tin["xT"].ap())

        # attention weights early
        WA = tc.tile_pool(name="wa_pool", bufs=1)
        WAp = WA.__enter__()
        wqk_s = WAp.tile([128, 2048], BF, tag="wqk_s")
        wv_s = WAp.tile([128, 8 * VW], BF, tag="wv_s")
        tri_s = WAp.tile([128, 2048], BF, tag="tri_s")
        dma(out=wqk_s[:], in_=tin["wqk"].ap())
        dma(out=wv_s[:], in_=tin["wv"].ap())
        dma(out=tri_s[:], in_=tin["tri"].ap())

        # ============ Phase 1: local LN(x) -> ag_in; AllGather ============
        with tc.tile_pool(name="ps_r1", bufs=1, space="PSUM") as PSR, \
             tc.tile_pool(name="tmp1", bufs=2) as TMP:
            t_sx = PSR.tile([1, 2 * TOK], F32, tag="sx1p")
            sx = t_sx[:, 0:TOK]
            sx2 = t_sx[:, TOK:2 * TOK]
            for kc in range(8):
                xb = TMP.tile([128, TOK], BF, tag="xb")
                v.tensor_copy(xb[:], xt[:, TOK * kc:TOK * (kc + 1)])
                te.matmul(sx, onesb[:], xb[:],
                          start=(kc == 0), stop=(kc == 7))
            for kc in range(8):
                xsq = TMP.tile([128, TOK], BF, tag="xsq")
                s.activation(xsq[:], xt[:, TOK * kc:TOK * (kc + 1)], Act.Square)
                te.matmul(sx2, onesb[:], xsq[:],
                          start=(kc == 0), stop=(kc == 7))
            # mu = sx/D ; var+eps = sx2/D - mu^2 + eps ; s1 = rsqrt
            v.tensor_scalar(rs("mu1"), sx, 1.0 / D, None, Alu.mult)
            v.tensor_tensor(rs("ra"), rs("mu1"), rs("mu1"), Alu.mult)
            v.tensor_scalar(rs("rb"), sx2, 1.0 / D, EPS, Alu.mult, Alu.add)
            v.tensor_tensor(rs("rb"), rs("rb"), rs("ra"), Alu.subtract)
            _quake_rsqrt(nc, rs("s1"), rs("rb"), rs("ra"), rs("rc"), rs("sc1"))
            mu_b = TMP.tile([128, TOK], F32, tag="mu_b", bufs=1)
            s_b = TMP.tile([128, TOK], F32, tag="s_b", bufs=1)
            g.partition_broadcast(mu_b[:], rs("mu1"))
            g.partition_broadcast(s_b[:], rs("s1"))
            tm = TMP.tile([128, TOK], F32, tag="tm")
            xlb = TMP.tile([128, 2048], BF, tag="xlb", bufs=1)
            for kc in range(8):
                v.tensor_tensor(tm[:], xt[:, TOK * kc:TOK * (kc + 1)],
                                mu_b[:], Alu.subtract)
                v.tensor_tensor(tm[:], tm[:], s_b[:], Alu.mult)
                v.tensor_scalar(xlb[:, TOK * kc:TOK * (kc + 1)], tm[:],
                                sm["lnw"][:, kc:kc + 1], sm["lnb"][:, kc:kc + 1],
                                Alu.mult, Alu.add)
            # ag_in[(kc*128+p), t] = xlb[p, kc*256+t]  (one strided DMA)
            dma(out=ag_in.ap().rearrange("(kc p) t -> p kc t", kc=8, p=128),
                in_=xlb[:].rearrange("p (kc t) -> p kc t", kc=8))
        g.collective_compute("AllGather", Alu.bypass, replica_groups=RG,
                             ins=[ag_in.ap()], outs=[ag_out.ap()])

        XA_cm = tc.tile_pool(name="xa_pool", bufs=1)
        XA = XA_cm.__enter__()
        xall = XA.tile([128, 16384], BF, tag="xall")
        # xall[p, kc*2048 + r*256 + t] = ag_out[(r*1024 + kc*128 + p), t]
        for kc in range(8):
            [nc.sync, nc.gpsimd][kc % 2].dma_start(
                out=xall[:, 2048 * kc:2048 * (kc + 1)]
                    .rearrange("p (r t) -> p r t", r=8),
                in_=ag_out.ap().rearrange("(r kc p) t -> kc p r t",
                                          r=8, kc=8, p=128)[kc])

        # big-weight prefetch (off the latency-critical sync queue).
        # wout/wgram/wepc/wff1/w2t cycle 4 shared 16KB slots in consumption
        # order (phase 5, phase 6 head, ff1 quarters, ff2 quarters).
        wout_s = WBIGp.tile([128, 8192], BF, tag="wbig", name="wout_s")
        wgram_s = WBIGp.tile([128, 8192], BF, tag="wbig", name="wgram_s")
        wepc_s = WBIGp.tile([128, 8192], BF, tag="wbig", name="wepc_s")
        wff1_t = [WBIGp.tile([128, 8192], BF, tag="wbig", name=f"wff1_{i}")
                  for i in range(4)]
        nc.scalar.dma_start(out=wout_s[:], in_=tin["wout"].ap())
        nc.gpsimd.dma_start(out=wgram_s[:], in_=tin["wgram"].ap())
        nc.scalar.dma_start(out=wepc_s[:, 0:2048], in_=tin["wepc"].ap())
        for i in range(4):
            [nc.gpsimd, nc.scalar, nc.gpsimd, nc.scalar][i].dma_start(
                out=wff1_t[i][:],
                in_=tin["wff1"].ap()[:, 8192 * i:8192 * (i + 1)])
        w2_t = [WBIGp.tile([128, 8192], BF, tag="wbig", name=f"w2_{i}")
                for i in range(4)]
        for i in range(4):
            [nc.scalar, nc.gpsimd, nc.scalar, nc.gpsimd][i].dma_start(
                out=w2_t[i][:],
                in_=tin["wff2"].ap()[:, 8192 * i:8192 * (i + 1)])

        # ============ Phase 2: QKV + V(+ent) ============
        with tc.tile_pool(name="ps_qk", bufs=2, space="PSUM") as PSQ, \
             tc.tile_pool(name="ps_ev", bufs=2, space="PSUM") as PSV, \
             tc.tile_pool(name="esb", bufs=1) as ESB:
            for of in range(2):
                for w in range(4):
                    ps = PSQ.tile([128, 512], F32, tag="psqk")
                    for kc in range(8):
                        te.matmul(
                            ps[:],
                            wqk_s[:, (of * 8 + kc) * 128:(of * 8 + kc + 1) * 128],
                            xall[:, 2048 * kc + 512 * w:2048 * kc + 512 * (w + 1)],
                            start=(kc == 0), stop=(kc == 7))
                    v.tensor_scalar(
                        qkT[:, 2048 * of + 512 * w:2048 * of + 512 * (w + 1)],
                        ps[:], sm["b_qk"][:, of:of + 1], None, Alu.add)

            elog = ESB.tile([128, 16], F32, tag="elog")
            es = ESB.tile([128, 16], F32, tag="es")
            for tch in range(16):
                psv = PSV.tile([128, VW], F32, tag="psv")
                for kc in range(8):
                    te.matmul(
                        psv[:],
                        xall[:, 2048 * kc + 128 * tch:2048 * kc + 128 * (tch + 1)],
                        wv_s[:, VW * kc:VW * (kc + 1)],
                        start=(kc == 0), stop=(kc == 7))
                vt = vaug[:, VW * tch:VW * (tch + 1)]
                v.tensor_tensor(vt, psv[:], bvb[:], Alu.add)
                v.tensor_copy(elog[:, tch:tch + 1], psv[:, 136:137])
            # es = clip(sigmoid(elog + ent_b), 0.1, 2.0)
            s.activation(es[:], elog[:], Act.Exp,
                         bias=cst[:, 0:1], scale=-1.0)
            v.tensor_scalar(es[:], es[:], 1.0, None, Alu.add)
            v.reciprocal(es[:], es[:])
            v.tensor_scalar(es[:], es[:], 0.1, 2.0, Alu.max, Alu.min)
            for tch in range(16):
                vt = vaug[:, VW * tch:VW * tch + 136]
                v.tensor_scalar(vt, vt, es[:, tch:tch + 1], None, Alu.mult)
                for lh in range(HPC):
                    v.memset(vaug[:, VW * tch + 68 * lh + 64:
                                  VW * tch + 68 * lh + 65], 1.0)
            if dbg:
                dma(out=dbg["d_es"].ap()[:, 0:16], in_=es[:])

        if dbg:
            with tc.tile_pool(name="dbgx", bufs=1) as DBGX:
                for qq in range(8):
                    cvx = DBGX.tile([128, 2048], F32, tag="cvx")
                    v.tensor_copy(cvx[:], xall[:, 2048 * qq:2048 * (qq + 1)])
                    dma(out=dbg["d_xall"].ap()[:, 2048 * qq:2048 * (qq + 1)],
                        in_=cvx[:])
        XA_cm.__exit__(None, None, None)

        # ============ Phase 3: attention ============
        with tc.tile_pool(name="ps_sc", bufs=2, space="PSUM") as PSS, \
             tc.tile_pool(name="ps_ao", bufs=2, space="PSUM") as PSA, \
             tc.tile_pool(name="att_sb", bufs=3) as ASB, \
             tc.tile_pool(name="ao_sb", bufs=8) as AOSB, \
             tc.tile_pool(name="den_sb", bufs=2) as DSB:
            for lh in range(HPC):
                den8 = DSB.tile([128, 512], F32, tag="den8")
                att_stash = []
                hq = qkT[64 * lh:64 * (lh + 1), 0:2048]
                hk = qkT[64 * lh:64 * (lh + 1), 2048:4096]
                for G in range(4):
                    nkb = 4 * G + 4
                    ao = PSA.tile([65, 512], F32, tag="ao")
                    for pj in range(nkb // 2):
                        ps = PSS.tile([128, 1024], F32, tag="ps_sc")
                        ex = ASB.tile([128, 1024], BF, tag="ex")
                        for half in range(2):
                            kb = 2 * pj + half
                            te.matmul(ps[:, 512 * half:512 * (half + 1)],
                                      hk[:, 128 * kb:128 * (kb + 1)],
                                      hq[:, 512 * G:512 * (G + 1)],
                                      start=True, stop=True)
                        s.activation(ex[:], ps[:], Act.Exp)
                        for half in range(2):
                            kb = 2 * pj + half
                            j = kb - 4 * G
                            exh = ex[:, 512 * half:512 * (half + 1)]
                            if 0 <= j < 4:
                                v.tensor_tensor(
                                    exh, exh, tri_s[:, 512 * j:512 * (j + 1)],
                                    Alu.mult)
                            te.matmul(
                                ao[:],
                                vaug[:, VW * kb + 68 * lh:
                                     VW * kb + 68 * lh + 65],
                                exh,
                                start=(kb == 0), stop=(kb == nkb - 1))
                    aos = AOSB.tile([65, 512], F32, tag="aos")
                    s.copy(aos[:], ao[0:65, :])
                    v.tensor_copy(den8[32 * G:32 * G + 1, :], aos[64:65, :])
                    att_stash.append((G, aos))
                v.reciprocal(den8[:], den8[:])
                for G, aos in att_stash:
                    rrow = ASB.tile([1, 512], BF, tag="rrow")
                    v.tensor_copy(rrow[0:1, :], den8[32 * G:32 * G + 1, :])
                    rbp = PSA.tile([64, 512], F32, tag="rbp")
                    te.matmul(rbp[:], onesr[:], rrow[:], start=True, stop=True)
                    v.tensor_tensor(
                        aosc[64 * lh:64 * (lh + 1), 512 * G:512 * (G + 1)],
                        aos[0:64, :], rbp[:], Alu.mult)

        WA.__exit__(None, None, None)

        # ============ Phase 4: AllToAll ============
        dma(out=a2a_in.ap().rearrange("(r p) t -> p r t", r=8, p=128),
            in_=aosc[:].rearrange("p (r t) -> p r t", r=8))
        g.collective_compute("AllToAll", Alu.bypass, replica_groups=RG,
                             ins=[a2a_in.ap()], outs=[a2a_out.ap()])
        dma(out=aofull[:].rearrange("p (r t) -> p r t", r=8),
            in_=a2a_out.ap().rearrange("(r p) t -> p r t", r=8, p=128))

        # ============ Phase 5: out proj + norm1 ============
        with tc.tile_pool(name="ps_out", bufs=3, space="PSUM") as PSO, \
             tc.tile_pool(name="ps_r2", bufs=1, space="PSUM") as PSR2, \
             tc.tile_pool(name="tmp2", bufs=2) as TMP2:
            for of in range(8):
                ps = PSO.tile([128, TOK], F32, tag="ps_out")
                for kc in range(8):
                    te.matmul(
                        ps[:],
                        wout_s[:, (of * 8 + kc) * 128:(of * 8 + kc + 1) * 128],
                        aofull[:, TOK * kc:TOK * (kc + 1)],
                        start=(kc == 0), stop=(kc == 7))
                v.scalar_tensor_tensor(xt[:, TOK * of:TOK * (of + 1)],
                                       ps[:], sm["b_out"][:, of:of + 1],
                                       xt[:, TOK * of:TOK * (of + 1)],
                                       Alu.add, Alu.add)
            _ln_full(nc, tc, TMP2, PSR2, rs, xt, x1f, x1b, ones32,
                     sm["n1w"], sm["n1b"])

        MID_cm.__exit__(None, None, None)

        # ============ Phase 6: ep path + h-stats + ff1 ============
        with tc.tile_pool(name="ps_h", bufs=2, space="PSUM") as PSH, \
             tc.tile_pool(name="ps_r3", bufs=1, space="PSUM") as PSR3, \
             tc.tile_pool(name="tmp3b", bufs=1) as TMP3b:
            # --- mean of h from x1 (tiny) ---
            t_muh = PSR3.tile([1, 2 * TOK], F32, tag="muhp")
            pmu = t_muh[:, 0:TOK]
            psh2 = t_muh[:, TOK:2 * TOK]
            for kc in range(8):
                te.matmul(pmu, sm["wsum"][:, kc:kc + 1],
                          x1b[:, TOK * kc:TOK * (kc + 1)],
                          start=(kc == 0), stop=(kc == 7))
            v.tensor_scalar(rs("muh"), pmu, 1.0, sc["bsum"], Alu.mult, Alu.add)

            # --- sum of squares of h via Gram matrix ---
            for of in range(8):
                of2 = of % 2
                if of2 == 0:
                    ybf = TMP3b.tile([128, 2 * TOK], BF, tag="ybf", bufs=2)
                    zbf = TMP3b.tile([128, 2 * TOK], BF, tag="zbf", bufs=2)
                ps = PSH.tile([128, TOK], F32, tag="ps_h")
                for kc in range(8):
                    te.matmul(
                        ps[:],
                        wgram_s[:, (of * 8 + kc) * 128:(of * 8 + kc + 1) * 128],
                        x1b[:, TOK * kc:TOK * (kc + 1)],
                        start=(kc == 0), stop=(kc == 7))
                s.activation(ybf[:, TOK * of2:TOK * (of2 + 1)], ps[:],
                             Act.Identity, bias=sm["c_lin"][:, of:of + 1])
                v.tensor_tensor(zbf[:, TOK * of2:TOK * (of2 + 1)],
                                ybf[:, TOK * of2:TOK * (of2 + 1)],
                                x1b[:, TOK * of:TOK * (of + 1)], Alu.mult)
                te.matmul(psh2, onesb[:], zbf[:, TOK * of2:TOK * (of2 + 1)],
                          start=(of == 0), stop=(of == 7))
            # var+eps = sh2/FF + btb/FF + eps - muh^2 ; S = rsqrt(.)/65
            v.tensor_tensor(rs("ra"), rs("muh"), rs("muh"), Alu.mult)
            v.tensor_scalar(rs("rb"), psh2, 1.0 / FF,
                            sc["btb"] / FF + EPS, Alu.mult, Alu.add)
            v.tensor_tensor(rs("rb"), rs("rb"), rs("ra"), Alu.subtract)
            _quake_rsqrt(nc, rs("Sh"), rs("rb"), rs("ra"), rs("rc"), rs("sc1"),
                         scale=1.0 / (1.0 + np.sqrt(FF)))
            v.tensor_tensor(rs("muS"), rs("muh"), rs("Sh"), Alu.mult)

            # --- ep gate path (contracted over D) ---
            t_se1 = PSR3.tile([1, TOK], F32, tag="se1p")
            t_se2 = PSR3.tile([1, TOK], F32, tag="se2p")
            se1 = t_se1[:]
            se2 = t_se2[:]
            t_pse2 = PSR3.tile([1, TOK], F32, tag="pse2p")
            pse2 = t_pse2[:]
            epb = TMP3b.tile([128, 2 * TOK], BF, tag="epb")
            epsq = TMP3b.tile([128, TOK], BF, tag="epsq")
            for of in range(2):
                ps = PSH.tile([128, TOK], F32, tag="ps_h")
                for kc in range(8):
                    te.matmul(
                        ps[:],
                        wepc_s[:, (of * 8 + kc) * 128:(of * 8 + kc + 1) * 128],
                        x1b[:, TOK * kc:TOK * (kc + 1)],
                        start=(kc == 0), stop=(kc == 7))
                s.activation(epb[:, TOK * of:TOK * (of + 1)], ps[:],
                             Act.Identity, bias=sm["b_epc"][:, of:of + 1])
                s.activation(epsq[:], ps[:], Act.Square,
                             bias=sm["b_epc"][:, of:of + 1])
                te.matmul(se1, onesb[:], epb[:, TOK * of:TOK * (of + 1)],
                          start=(of == 0), stop=(of == 1))
                te.matmul(se2, onesb[:], epsq[:],
                          start=(of == 0), stop=(of == 1))
            v.tensor_scalar(rs("mue"), se1, 1.0 / D16, None, Alu.mult)
            v.tensor_tensor(rs("ra"), rs("mue"), rs("mue"), Alu.mult)
            v.tensor_scalar(rs("rb"), se2, 1.0 / D16, EPS, Alu.mult, Alu.add)
            v.tensor_tensor(rs("rb"), rs("rb"), rs("ra"), Alu.subtract)
            _quake_rsqrt(nc, rs("se"), rs("rb"), rs("ra"), rs("rc"), rs("sc1"))
            mue_b = TMP3b.tile([128, TOK], F32, tag="mue_b")
            see_b = TMP3b.tile([128, TOK], F32, tag="see_b")
            g.partition_broadcast(mue_b[:], rs("mue"))
            g.partition_broadcast(see_b[:], rs("se"))
            relub = TMP3b.tile([128, 2 * TOK], BF, tag="relub")
            tm3 = TMP3b.tile([128, TOK], F32, tag="tm3")
            for of in range(2):
                v.tensor_tensor(tm3[:], epb[:, TOK * of:TOK * (of + 1)],
                                mue_b[:], Alu.subtract)
                v.tensor_tensor(tm3[:], tm3[:], see_b[:], Alu.mult)
                s.activation(relub[:, TOK * of:TOK * (of + 1)], tm3[:],
                             Act.Relu, bias=sm["eplb"][:, of:of + 1],
                             scale=sm["eplw"][:, of:of + 1])
            for of in range(2):
                te.matmul(pse2, sm["wep2"][:, of:of + 1],
                          relub[:, TOK * of:TOK * (of + 1)],
                          start=(of == 0), stop=(of == 1))
            # em = 1 + 0.1*sigmoid(pse2 + ep2_b)
            s.activation(rs("em"), pse2, Act.Exp, bias=cst[0:1, 1:2], scale=-1.0)
            v.tensor_scalar(rs("em"), rs("em"), 1.0, None, Alu.add)
            v.reciprocal(rs("em"), rs("em"))
            v.tensor_scalar(rs("em"), rs("em"), 0.1, 1.0, Alu.mult, Alu.add)

            # --- ff1 ---
            hb = HBp.tile([128, 8192], BF, tag="hb")
            for c in range(32):
                wt = wff1_t[c // 8]
                cl = c % 8
                ps = PSH.tile([128, TOK], F32, tag="ps_h")
                for kc in range(8):
                    te.matmul(ps[:],
                              wt[:, (cl * 8 + kc) * 128:(cl * 8 + kc + 1) * 128],
                              x1b[:, TOK * kc:TOK * (kc + 1)],
                              start=(kc == 0), stop=(kc == 7))
                s.activation(hb[:, TOK * c:TOK * (c + 1)], ps[:], Act.Identity,
                             bias=sm["b_ff1"][:, c:c + 1])

            # broadcast per-token spline rows
            Sh_b = TMP3.tile([128, TOK], F32, tag="Sh_b")
            muS_b = TMP3.tile([128, TOK], F32, tag="muS_b")
            em_b = TMP3.tile([128, TOK], F32, tag="em_b")
            g.partition_broadcast(Sh_b[:], rs("Sh"))
            g.partition_broadcast(muS_b[:], rs("muS"))
            g.partition_broadcast(em_b[:], rs("em"))
            Srep = TMP3.tile([128, 2048], BF, tag="Srep")
            muSrep = TMP3.tile([128, 2048], BF, tag="muSrep")
            emrep = TMP3.tile([128, 2048], BF, tag="emrep")
            for src8, t8 in ((Sh_b, Srep), (muS_b, muSrep), (em_b, emrep)):
                v.tensor_copy(t8[:], src8[:].unsqueeze(1)
                              .to_broadcast((128, 8, TOK)))
        # ============ Phase 7: spline + ff2 interleaved ============
        if True:
            a_q, d_q, s2, c0p, c3 = (spl["a"], spl["d"], spl["s2"],
                                     spl["c0p"], spl["c3"])
            op_q = Alu.add if s2 > 0 else Alu.subtract
            with tc.tile_pool(name="spl_sb", bufs=2) as SPL:
              r2 = SPL.tile([128, 8 * TOK], F32, tag="r2", bufs=1)
              with tc.tile_pool(name="ps_f2", bufs=1, space="PSUM") as PSF:
                psF = [PSF.tile([128, TOK], F32, tag=f"psF{i}", name=f"psF{i}")
                       for i in range(8)]
                DBGU_cm = tc.tile_pool(name="dbgu", bufs=1) if dbg else None
                DBGU = DBGU_cm.__enter__() if dbg else None
                for gi in range(4):
                    w2t = w2_t[gi]
                    hbs = hb[:, 2048 * gi:2048 * (gi + 1)]
                    u = SPL.tile([128, 2048], BF, tag="u")
                    q = SPL.tile([128, 2048], BF, tag="q")
                    t3 = SPL.tile([128, 2048], BF, tag="t3")
                    acc = SPL.tile([128, 2048], BF, tag="acc")
                    actt = SPL.tile([128, 2048], BF, tag="actt")
                    v.tensor_tensor(u[:], hbs, Srep[:], Alu.mult)
                    v.tensor_tensor(u[:], u[:], muSrep[:], Alu.subtract)
                    s.activation(q[:], u[:], Act.Square, bias=cst[:, 2:3], scale=a_q)
                    s.activation(t3[:], u[:], Act.Abs)
                    v.scalar_tensor_tensor(acc[:], t3[:], c3, q[:],
                                           Alu.mult, op_q)
                    v.tensor_scalar(acc[:], acc[:], c0p, None, Alu.add)
                    v.tensor_tensor(acc[:], acc[:], emrep[:], Alu.mult)
                    v.tensor_scalar(actt[:], acc[:], 1.0, -1.0,
                                    Alu.min, Alu.max)
                    for kc8 in range(8):
                        kc = 8 * gi + kc8
                        for of in range(8):
                            te.matmul(
                                psF[of][:],
                                w2t[:, (kc8 * 8 + of) * 128:(kc8 * 8 + of + 1) * 128],
                                actt[:, TOK * kc8:TOK * (kc8 + 1)],
                                start=(kc == 0), stop=(kc == 31))
                    if dbg:
                        cvu = DBGU.tile([128, 2048], F32, tag="cvu")
                        v.tensor_copy(cvu[:], u[:])
                        dma(out=dbg["d_u"].ap()[:, 2048 * gi:2048 * (gi + 1)],
                            in_=cvu[:])
                        cva = DBGU.tile([128, 2048], F32, tag="cva")
                        v.tensor_copy(cva[:], actt[:])
                        dma(out=dbg["d_actt"].ap()[:, 2048 * gi:2048 * (gi + 1)],
                            in_=cva[:])

                if dbg:
                    DBGU_cm.__exit__(None, None, None)
                # ff2 epilogue (inside PSF scope)
                for of in range(8):
                    v.scalar_tensor_tensor(
                        r2[:, TOK * of:TOK * (of + 1)],
                        psF[of][:],
                        sm["b_ff2"][:, of:of + 1],
                        x1f[:, TOK * of:TOK * (of + 1)],
                        Alu.add, Alu.add)
              # ============ Phase 8: norm2 ============
              with tc.tile_pool(name="ps_r4", bufs=1, space="PSUM") as PSR4, \
                   tc.tile_pool(name="tmp4", bufs=2) as TMP4:
                    t_sx4 = PSR4.tile([1, 2 * TOK], F32, tag="lnsx4")
                    sx4 = t_sx4[:, 0:TOK]
                    sx42 = t_sx4[:, TOK:2 * TOK]
                    for of in range(8):
                        te.matmul(sx4, ones32[:], r2[:, TOK * of:TOK * (of + 1)],
                                  start=(of == 0), stop=(of == 7))
                    xsq4 = TMP4.tile([128, TOK], F32, tag="xsq4")
                    for of in range(8):
                        s.activation(xsq4[:], r2[:, TOK * of:TOK * (of + 1)],
                                     Act.Square)
                        te.matmul(sx42, ones32[:], xsq4[:],
                                  start=(of == 0), stop=(of == 7))
                    if dbg:
                        dma(out=dbg["d_r2"].ap(), in_=r2[:])
                    v.tensor_scalar(rs("m1"), sx4, 1.0 / D, None, Alu.mult)
                    v.tensor_tensor(rs("ra"), rs("m1"), rs("m1"), Alu.mult)
                    v.tensor_scalar(rs("rb"), sx42, 1.0 / D, EPS,
                                    Alu.mult, Alu.add)
                    v.tensor_tensor(rs("rb"), rs("rb"), rs("ra"), Alu.subtract)
                    _quake_rsqrt(nc, rs("m2"), rs("rb"), rs("ra"), rs("rc"),
                                 rs("sc1"))
                    mu4_b = TMP4.tile([128, TOK], F32, tag="mu4_b", bufs=1)
                    s4_b = TMP4.tile([128, TOK], F32, tag="s4_b", bufs=1)
                    g.partition_broadcast(mu4_b[:], rs("m1"))
                    g.partition_broadcast(s4_b[:], rs("m2"))
                    tm4 = TMP4.tile([128, TOK], F32, tag="tm4")
                    for of in range(8):
                        v.tensor_tensor(tm4[:], r2[:, TOK * of:TOK * (of + 1)],
                                        mu4_b[:], Alu.subtract)
                        v.tensor_tensor(tm4[:], tm4[:], s4_b[:], Alu.mult)
                        v.tensor_scalar(xt[:, TOK * of:TOK * (of + 1)], tm4[:],
                                        sm["n2w"][:, of:of + 1],
                                        sm["n2b"][:, of:of + 1],
                                        Alu.mult, Alu.add)
                        if of % 4 == 3:
                            dma(out=t_out.ap()[:, TOK * (of - 3):TOK * (of + 1)],
                                in_=xt[:, TOK * (of - 3):TOK * (of + 1)])
            HB_cm.__exit__(None, None, None)
            WBIG.__exit__(None, None, None)

        TMP3_cm.__exit__(None, None, None)
        # (HB/MID closed above)
        if dbg:
            with tc.tile_pool(name="dbgp", bufs=1) as DBG:
                def dump(name, tile_ap, width):
                    nch = max(1, width // 2048)
                    w = width // nch
                    for qq in range(nch):
                        cv = DBG.tile([128, w], F32, tag="cv",
                                      name=f"cv{name}{qq}")
                        v.tensor_copy(cv[:], tile_ap[:, w * qq:w * (qq + 1)])
                        dma(out=dbg[name].ap()[:, w * qq:w * (qq + 1)],
                            in_=cv[:])
                dump("d_qkT", qkT[:], 4096)
                dump("d_vaug", vaug[:], 16 * VW)
                dump("d_aofull", aofull[:], 8 * TOK)
                dma(out=dbg["d_x1f"].ap(), in_=x1f[:])
                dma(out=dbg["d_rows"].ap()[:, 0:NROW * TOK], in_=rows[:])


def _ln_full(nc, tc, TMP, PSR, rs, src, dstf, dstb, ones32, wcol, bcol):
    v, s, g, te = nc.vector, nc.scalar, nc.gpsimd, nc.tensor
    T = TOK
    t_sx = PSR.tile([1, 2 * T], F32, tag="lnsxp")
    sx = t_sx[:, 0:T]
    sx2 = t_sx[:, T:2 * T]
    for kc in range(8):
        te.matmul(sx, ones32[:], src[:, T * kc:T * (kc + 1)],
                  start=(kc == 0), stop=(kc == 7))
    xsq = TMP.tile([128, T], F32, tag="lnxsq")
    for kc in range(8):
        s.activation(xsq[:], src[:, T * kc:T * (kc + 1)], Act.Square)
        te.matmul(sx2, ones32[:], xsq[:], start=(kc == 0), stop=(kc == 7))
    v.tensor_scalar(rs("m1"), sx, 1.0 / D, None, Alu.mult)
    v.tensor_tensor(rs("ra"), rs("m1"), rs("m1"), Alu.mult)
    v.tensor_scalar(rs("rb"), sx2, 1.0 / D, EPS, Alu.mult, Alu.add)
    v.tensor_tensor(rs("rb"), rs("rb"), rs("ra"), Alu.subtract)
    _quake_rsqrt(nc, rs("m2"), rs("rb"), rs("ra"), rs("rc"), rs("sc1"))
    mu_b = TMP.tile([128, T], F32, tag="lnmu_b")
    s_b = TMP.tile([128, T], F32, tag="lns_b")
    g.partition_broadcast(mu_b[:], rs("m1"))
    g.partition_broadcast(s_b[:], rs("m2"))
    tm = TMP.tile([128, T], F32, tag="lntm")
    for kc in range(8):
        v.tensor_tensor(tm[:], src[:, T * kc:T * (kc + 1)], mu_b[:],
                        Alu.subtract)
        v.tensor_tensor(tm[:], tm[:], s_b[:], Alu.mult)
        v.tensor_scalar(dstf[:, T * kc:T * (kc + 1)], tm[:],
                        wcol[:, kc:kc + 1], bcol[:, kc:kc + 1],
                        Alu.mult, Alu.add)
        if dstb is not None:
            s.activation(dstb[:, T * kc:T * (kc + 1)], tm[:], Act.Identity,
                         bias=bcol[:, kc:kc + 1], scale=wcol[:, kc:kc + 1])


# ----------------------------------------------------------------------------
# Entry point
# ----------------------------------------------------------------------------

def kernel(**inputs):
    in_maps, sc = _prepare_inputs(inputs)
    key = hashlib.sha256(
        repr((sc["ent_b"], sc["ep2_b"], sc["bsum"], sc["btb"],
              sorted(sc["spl"].items()))).encode()
    ).hexdigest()
    if key not in _prog_cache:
        _prog_cache[key] = _build_program(sc)
    nc = _prog_cache[key]
    res = bass_utils.run_bass_kernel_spmd(nc, in_maps,
                                          core_ids=list(range(NCORES)))
    out = np.empty((1, S, D), np.float32)
    for c in range(NCORES):
        oc = np.asarray(res.results[c]["out"], np.float32)   # [128, 8*TOK]
        ot = oc.reshape(128, 8, TOK).transpose(1, 0, 2).reshape(D, TOK)
        out[0, c * TOK:(c + 1) * TOK, :] = ot.T
    return out


# revision 42
# speedup vs baseline: 1.0126x; 1.0126x over previous
"""Trainium2 8-core kernel for nn_EnhancedTransformerBlock (v2).

SPMD: identical program on all 8 cores, only in_maps data differs.
  - Sequence-sharded everywhere except attention: core c owns tokens
    [256c, 256c+256), activations in T-layout [feature, token].
  - Attention head-sharded (2 of 16 heads per core, full sequence).
    AllGather of ln(x) (bf16) before QKV; AllToAll of per-head attention
    outputs back to sequence sharding. A dummy tiny collective is issued
    first to absorb the NRT bootstrap barrier during input DMA/LN.
  - All GEMMs bf16 (weights pre-transposed/packed host-side), fp32 PSUM.
  - Softmax: temperature (1/0.1) and 1/sqrt(hd) folded into Wq; unshifted
    exp; denominator via ones-column appended to V; causal masking via
    triangle-mask multiply on diagonal blocks. The entropy gate (ent) is
    folded into the V GEMM as a 137th output column.
  - Only exp-set scalar activations are used (Exp/Abs/Identity/Square/
    Relu/Copy all live in the exp_and_others ACT table set) -> exactly one
    ACT_TABLE_LOAD. All rsqrt row math uses a Quake-style bit hack + two
    Newton steps on the vector engine.
  - FFN: mean/var of h computed directly from x1 via host-precomputed
    folds (row-sum vector for the mean; Gram matrix G = W1^T W1 for the
    sum of squares), so the spline scale S = rsqrt(var+eps)/(1+sqrt(FF))
    is ready before ff1 finishes. (1+norm) == 1+sqrt(FF) to ~1e-6 rel.
  - ep gate path contracted over D instead of FF via Wc = ep1_w @ ff1_w
    (host precompute), so it runs in parallel with ff1.
  - Spline activation g(u) approximated by a 4-term basis [1, u, u^2,
    |u|] LSQ-fit on |u|<=0.15 (|u| < 0.08 in practice); the quadratic
    part is computed with one scalar-engine Square via completing the
    square. ff2 is interleaved per 8-chunk group with the spline.
"""

import hashlib
import numpy as np

from concourse import bacc, tile, mybir
from concourse import bass_utils

dt = mybir.dt
BF = dt.bfloat16
F32 = dt.float32
I32 = dt.int32
NPBF = dt.np(BF)
Alu = mybir.AluOpType
Act = mybir.ActivationFunctionType

NCORES = 8
S = 2048
D = 1024
H = 16
HD = 64
FF = 4096
D16 = 256
TOK = S // NCORES            # 256 tokens per core
HPC = H // NCORES            # 2 heads per core
EPS = 1e-6
UDOM = 0.15                  # spline fit domain |u| <= UDOM
VW = 137                     # augmented V width: 2*68 + ent column
QK_C = 0x5F3759E0            # quake magic + 1 (for the xor/add form)

_prog_cache = {}


# ----------------------------------------------------------------------------
# Host-side: spline fit
# ----------------------------------------------------------------------------

def _g_exact(u, knots, spl_w):
    d = np.abs(u[:, None] - knots[None, :])
    d = d / (d.max(-1, keepdims=True) + EPS)
    a = -5.0 * d
    a = a - a.max(-1, keepdims=True)
    e = np.exp(a)
    p = e / e.sum(-1, keepdims=True)
    return (p * spl_w).sum(-1)


def _fit_spline(knots, spl_w):
    """LSQ fit of g(u) on [-UDOM, UDOM] with basis [1, u, u^2, |u|].
    Returns dict with the square-trick constants."""
    k = np.asarray(knots, np.float64)
    w = np.asarray(spl_w, np.float64)
    u = np.linspace(-UDOM, UDOM, 20001)
    B = np.stack([np.ones_like(u), u, u * u, np.abs(u)], -1)
    y = _g_exact(u, k, w)
    c, *_ = np.linalg.lstsq(B, y, rcond=None)
    err = float(np.abs(B @ c - y).max())
    c0, c1, c2, c3 = (float(v) for v in c)
    s2 = 1.0 if c2 >= 0 else -1.0
    a = max(np.sqrt(abs(c2)), 1e-3)
    dq = c1 / (2.0 * s2 * a)
    c0p = c0 - s2 * dq * dq + s2 * a * a * 0.0
    # residual error from the a-floor when |c2| tiny:
    # (a^2 - |c2|) * u^2 <= (1e-6)*UDOM^2 -- negligible.
    return {"a": float(a), "d": float(dq), "s2": s2, "c0p": float(c0p),
            "c3": c3, "fit_err": err}


# ----------------------------------------------------------------------------
# Host-side: weight packing
# ----------------------------------------------------------------------------

def _pack_lhsT(w_t, n_of, n_kc, kc_major=False):
    """w_t: [K_total, M_total] ([in, out]) -> [128, n_of*n_kc*128].
    of-major tile order by default; kc-major if requested."""
    K_total, M_total = w_t.shape
    assert K_total == n_kc * 128 and M_total == n_of * 128
    out = np.empty((128, n_of * n_kc * 128), np.float32)
    for of in range(n_of):
        for kc in range(n_kc):
            idx = (kc * n_of + of) if kc_major else (of * n_kc + kc)
            out[:, idx * 128:(idx + 1) * 128] = \
                w_t[kc * 128:(kc + 1) * 128, of * 128:(of + 1) * 128]
    return np.ascontiguousarray(out)


def _col_pack(vec, n_chunks):
    return np.ascontiguousarray(
        np.asarray(vec, np.float32).reshape(n_chunks, 128).T)


def _make_tri_masks():
    out = np.zeros((128, 4 * 512), np.float32)
    for j in range(4):
        kk = np.arange(128)[:, None] + 128 * j
        q = np.arange(512)[None, :]
        out[:, 512 * j:512 * (j + 1)] = (kk <= q).astype(np.float32)
    return out


def _prepare_inputs(inputs):
    f = lambda k: np.asarray(inputs[k], np.float32)
    x = f("x").reshape(S, D)
    qkv_w, qkv_b = f("qkv_w"), f("qkv_b")
    out_w, out_b = f("out_w") * 0.1, f("out_b") * 0.1
    ff1_w, ff1_b = f("ff1_w"), f("ff1_b")
    ff2_w, ff2_b = f("ff2_w"), f("ff2_b")
    ep1_w, ep1_b = f("ep1_w"), f("ep1_b")
    ep2_w, ep2_b = f("ep2_w"), f("ep2_b")
    ent_w, ent_b = f("ent_w"), f("ent_b")

    temp = (1.0 / np.sqrt(np.float32(HD))) / 0.1   # 1.25
    wq = qkv_w[0:D] * temp
    wk = qkv_w[D:2 * D]
    wv = qkv_w[2 * D:3 * D]
    bq = qkv_b[0:D] * temp
    bk = qkv_b[D:2 * D]
    bv = qkv_b[2 * D:3 * D]

    spl = _fit_spline(f("knots"), f("spl_w"))

    # ep-path fold: h @ ep1_w.T = x1 @ (ep1_w @ ff1_w).T + ep1_w @ ff1_b
    wc = (ep1_w.astype(np.float64) @ ff1_w.astype(np.float64)).astype(np.float32)
    bc = ep1_b + ep1_w @ ff1_b
    # mean of h fold
    wsum = ff1_w.sum(0) / FF                        # [D]
    bsum = float(ff1_b.sum()) / FF
    # sum-of-squares fold: G = W1^T W1, linear term, const term
    G = (ff1_w.T.astype(np.float64) @ ff1_w.astype(np.float64)).astype(np.float32)
    c_lin = 2.0 * (ff1_b @ ff1_w)                   # [D]
    btb = float(ff1_b @ ff1_b)

    # consolidated f32 constants: one DMA instead of ~20
    cpack = np.concatenate([
        np.ones((128, 1), np.float32),      # ones32      0:1
        _col_pack(out_b, 8),                # b_out       1:9
        _col_pack(ff1_b, 32),               # b_ff1       9:41
        _col_pack(ff2_b, 8),                # b_ff2      41:49
        _col_pack(bc, 2),                   # b_epc      49:51
        _col_pack(c_lin, 8),                # c_lin      51:59
        _col_pack(f("ln_attn_w"), 8),       # lnw        59:67
        _col_pack(f("ln_attn_b"), 8),       # lnb        67:75
        _col_pack(f("norm1_w"), 8),         # n1w        75:83
        _col_pack(f("norm1_b"), 8),         # n1b        83:91
        _col_pack(f("norm2_w"), 8),         # n2w        91:99
        _col_pack(f("norm2_b"), 8),         # n2b        99:107
        _col_pack(f("ep_ln_w"), 2),         # eplw      107:109
        _col_pack(f("ep_ln_b"), 2),         # eplb      109:111
    ], 1)
    bpack = np.concatenate([
        np.ones((128, 1), np.float32),      # onesb       0:1
        _col_pack(wsum, 8),                 # wsum        1:9
        np.ascontiguousarray(ep2_w.reshape(2, 128).T),  # wep2 9:11
    ], 1).astype(NPBF)
    shared = {
        "tri": _make_tri_masks().astype(NPBF),
        "cpack": cpack,
        "bpack": bpack,
        "wff1": _pack_lhsT(ff1_w.T, 32, 8).astype(NPBF),
        "wff2": _pack_lhsT(ff2_w.T, 8, 32, kc_major=True).astype(NPBF),
        "wepc": _pack_lhsT(wc.T, 2, 8).astype(NPBF),
        "wgram": _pack_lhsT(G, 8, 8).astype(NPBF),
        "wout": _pack_lhsT(out_w.T, 8, 8).astype(NPBF),
    }

    scalars = {
        "ent_b": float(ent_b.reshape(-1)[0]),
        "ep2_b": float(ep2_b.reshape(-1)[0]),
        "bsum": bsum,
        "btb": btb,
        "spl": spl,
    }

    in_maps = []
    for c in range(NCORES):
        m = dict(shared)
        xc = x[c * TOK:(c + 1) * TOK]                        # [256, D]
        xT = np.ascontiguousarray(xc.T)                      # [D, 256]
        m["xT"] = np.ascontiguousarray(
            xT.reshape(8, 128, TOK).transpose(1, 0, 2).reshape(128, 8 * TOK))
        h0 = c * HPC
        wq_c = wq[h0 * HD:(h0 + HPC) * HD]                   # [128, D]
        wk_c = wk[h0 * HD:(h0 + HPC) * HD]
        wqk_t = np.concatenate([wq_c, wk_c], 0).T            # [D, 256]
        m["wqk"] = _pack_lhsT(wqk_t, 2, 8).astype(NPBF)
        m["b_qk"] = np.ascontiguousarray(np.stack(
            [bq[h0 * HD:(h0 + HPC) * HD],
             bk[h0 * HD:(h0 + HPC) * HD]], -1).astype(np.float32))
        wv_c = wv[h0 * HD:(h0 + HPC) * HD].T                 # [D, 128]
        wva = np.zeros((D, VW), np.float32)
        bva = np.zeros((1, VW), np.float32)
        for lh in range(HPC):
            wva[:, 68 * lh:68 * lh + 64] = wv_c[:, 64 * lh:64 * lh + 64]
            bva[0, 68 * lh:68 * lh + 64] = \
                bv[(h0 + lh) * HD:(h0 + lh + 1) * HD]
        wva[:, 136] = ent_w[0]                               # ent gate column
        m["wv"] = np.ascontiguousarray(
            wva.reshape(8, 128, VW).transpose(1, 0, 2).reshape(128, 8 * VW)
        ).astype(NPBF)
        m["bvb"] = np.ascontiguousarray(np.tile(bva, (128, 1)))
        in_maps.append(m)

    return in_maps, scalars


# ----------------------------------------------------------------------------
# Device program helpers
# ----------------------------------------------------------------------------

def _quake_rsqrt(nc, out, v, t_i, y_f, t2_f, scale=1.0):
    """out = scale / sqrt(v) elementwise on f32 row APs, vector engine only.
    t_i (int32-viewable f32 tile), y_f, t2_f are scratch APs, same shape."""
    v_ = nc.vector
    # y0 bits = C - (v_bits >> 1)  ==  ((v>>1) ^ ~0) + (C+1)
    v_.tensor_scalar(t_i.bitcast(I32), v.bitcast(I32), 1, -1,
                     Alu.arith_shift_right, Alu.bitwise_xor)
    v_.tensor_scalar(y_f.bitcast(I32), t_i.bitcast(I32), QK_C, None, Alu.add)
    # newton 1: y = y*(1.5 - 0.5*v*y*y)
    v_.tensor_tensor(t_i, y_f, y_f, Alu.mult)
    v_.tensor_tensor(t_i, t_i, v, Alu.mult)
    v_.tensor_scalar(t2_f, t_i, -0.5, 1.5, Alu.mult, Alu.add)
    v_.tensor_tensor(y_f, t2_f, y_f, Alu.mult)
    # newton 2 (scaled): out = scale * y*(1.5 - 0.5*v*y*y)
    v_.tensor_tensor(t_i, y_f, y_f, Alu.mult)
    v_.tensor_tensor(t_i, t_i, v, Alu.mult)
    v_.tensor_scalar(t2_f, t_i, -0.5 * scale, 1.5 * scale, Alu.mult, Alu.add)
    v_.tensor_tensor(out, t2_f, y_f, Alu.mult)


def _build_program(sc):
    nc = bacc.Bacc("TRN2", target_bir_lowering=False, debug=False,
                   num_devices=NCORES)

    def din(name, shape, dtype):
        return nc.dram_tensor(name, list(shape), dtype, kind="ExternalInput")

    tin = {
        "xT": din("xT", (128, 8 * TOK), F32),
        "wqk": din("wqk", (128, 2048), BF),
        "wv": din("wv", (128, 8 * VW), BF),
        "wout": din("wout", (128, 8192), BF),
        "wff1": din("wff1", (128, 32768), BF),
        "wff2": din("wff2", (128, 32768), BF),
        "wepc": din("wepc", (128, 2048), BF),
        "wgram": din("wgram", (128, 8192), BF),
        "tri": din("tri", (128, 2048), BF),
        "cpack": din("cpack", (128, 111), F32),
        "bpack": din("bpack", (128, 11), BF),
        "b_qk": din("b_qk", (128, 2), F32),
        "bvb": din("bvb", (128, VW), F32),
    }
    t_out = nc.dram_tensor("out", [128, 8 * TOK], F32, kind="ExternalOutput")
    import os
    dbg = {}
    if os.environ.get("KDEBUG", "0") == "1":
        for nm, shape in (("d_xall", (128, 16384)), ("d_qkT", (128, 4096)),
                          ("d_vaug", (128, 16 * VW)), ("d_es", (128, 16)),
                          ("d_aosc", (128, 2048)), ("d_aofull", (128, 8 * TOK)),
                          ("d_x1f", (128, 8 * TOK)), ("d_hb", (128, 8192)),
                          ("d_actt", (128, 8192)), ("d_rows", (1, 16 * TOK)),
                          ("d_u", (128, 8192)), ("d_r2", (128, 8 * TOK))):
            dbg[nm] = nc.dram_tensor(nm, list(shape), F32, kind="ExternalOutput")
    ag_in = nc.dram_tensor("ag_in", [1024, TOK], BF, kind="Internal")
    ag_out = nc.dram_tensor("ag_out", [8192, TOK], BF, kind="Internal",
                            addr_space="Shared")
    a2a_in = nc.dram_tensor("a2a_in", [1024, TOK], BF, kind="Internal")
    a2a_out = nc.dram_tensor("a2a_out", [1024, TOK], BF, kind="Internal")

    with tile.TileContext(nc) as tc:
        _emit(nc, tc, tin, t_out, ag_in, ag_out, a2a_in, a2a_out, sc, dbg)
    nc.compile()
    return nc


def _emit(nc, tc, tin, t_out, ag_in, ag_out, a2a_in, a2a_out, sc, dbg):
    v = nc.vector
    s = nc.scalar
    g = nc.gpsimd
    te = nc.tensor
    dma = nc.sync.dma_start
    spl = sc["spl"]
    RG = [list(range(NCORES))]

    with tc.tile_pool(name="persist", bufs=1) as P, \
         tc.tile_pool(name="consts", bufs=1) as C, \
         tc.tile_pool(name="rows", bufs=1) as R:

        # persistent tiles
        onesr = P.tile([1, 64], BF, tag="onesr")
        xt = P.tile([128, 8 * TOK], F32, tag="xt")
        x1f = P.tile([128, 8 * TOK], F32, tag="x1f")
        x1b = P.tile([128, 8 * TOK], BF, tag="x1b")

        # constants: two packed DMAs + slice views
        cpk = C.tile([128, 111], F32, tag="cpk")
        bpk = C.tile([128, 11], BF, tag="bpk")
        bqk = C.tile([128, 2], F32, tag="bqk")
        bvb = C.tile([128, VW], F32, tag="bvb")
        dma(out=cpk[:], in_=tin["cpack"].ap())
        dma(out=bpk[:], in_=tin["bpack"].ap())
        dma(out=bqk[:], in_=tin["b_qk"].ap())
        dma(out=bvb[:], in_=tin["bvb"].ap())
        _coff = {"ones32": (0, 1), "b_out": (1, 9), "b_ff1": (9, 41),
                 "b_ff2": (41, 49), "b_epc": (49, 51), "c_lin": (51, 59),
                 "lnw": (59, 67), "lnb": (67, 75), "n1w": (75, 83),
                 "n1b": (83, 91), "n2w": (91, 99), "n2b": (99, 107),
                 "eplw": (107, 109), "eplb": (109, 111)}
        sm = {nm: cpk[:, a:b] for nm, (a, b) in _coff.items()}
        sm["onesb"] = bpk[:, 0:1]
        sm["wsum"] = bpk[:, 1:9]
        sm["wep2"] = bpk[:, 9:11]
        sm["b_qk"] = bqk[:]
        ones32, onesb = sm["ones32"], sm["onesb"]
        cst = C.tile([128, 3], F32, tag="cst")
        v.memset(cst[:, 0:1], -sc["ent_b"])
        v.memset(cst[:, 1:2], -sc["ep2_b"])
        v.memset(cst[:, 2:3], sc["spl"]["d"])

        v.memset(onesr[:], 1.0)

        # pool opens (LIFO close order: XA, WA, MID, HB, WBIG, TMP3)
        TMP3_cm = tc.tile_pool(name="tmp3", bufs=1)
        TMP3 = TMP3_cm.__enter__()
        W3 = tc.tile_pool(name="w3_pool", bufs=1)
        W3p = W3.__enter__()
        WF1 = tc.tile_pool(name="wf1_pool", bufs=6)
        WF1p = WF1.__enter__()
        WF2 = tc.tile_pool(name="wf2_pool", bufs=2)
        WF2p = WF2.__enter__()
        HB_cm = tc.tile_pool(name="hb_pool", bufs=1)
        HBp = HB_cm.__enter__()
        MID_cm = tc.tile_pool(name="mid_pool", bufs=1)
        MIDp = MID_cm.__enter__()
        qkT = MIDp.tile([128, 4096], BF, tag="qkT")
        vaug = MIDp.tile([128, 16 * VW], BF, tag="vaug")
        aosc = MIDp.tile([128, 2048], BF, tag="aoshare", name="aosc")
        aofull = MIDp.tile([128, 8 * TOK], BF, tag="aoshare", name="aofull")

        # rows: [1, TOK] f32 rows packed in one tile; index by name
        NROW = 14
        rows = R.tile([1, NROW * TOK], F32, tag="rows")
        _r = {}
        for i, nm in enumerate(("mu1", "s1", "ra", "rb", "rc",
                                "muh", "Sh", "muS", "em",
                                "mue", "se", "m1", "m2", "sc1")):
            _r[nm] = rows[0:1, i * TOK:(i + 1) * TOK]
        rs = lambda nm: _r[nm]

        dma(out=xt[:], in_=tin["xT"].ap())

        # attention weights early
        WA = tc.tile_pool(name="wa_pool", bufs=1)
        WAp = WA.__enter__()
        wqk_s = WAp.tile([128, 2048], BF, tag="wqk_s")
        wv_s = WAp.tile([128, 8 * VW], BF, tag="wv_s")
        tri_s = WAp.tile([128, 2048], BF, tag="tri_s")
        dma(out=wqk_s[:], in_=tin["wqk"].ap())
        dma(out=wv_s[:], in_=tin["wv"].ap())
        dma(out=tri_s[:], in_=tin["tri"].ap())

        # ============ Phase 1: local LN(x) -> ag_in; AllGather ============
        with tc.tile_pool(name="ps_r1", bufs=1, space="PSUM") as PSR, \
             tc.tile_pool(name="tmp1", bufs=2) as TMP:
            t_sx = PSR.tile([1, 2 * TOK], F32, tag="sx1p")
            sx = t_sx[:, 0:TOK]
            sx2 = t_sx[:, TOK:2 * TOK]
            for kc in range(8):
                xb = TMP.tile([128, TOK], BF, tag="xb")
                v.tensor_copy(xb[:], xt[:, TOK * kc:TOK * (kc + 1)])
                te.matmul(sx, onesb[:], xb[:],
                          start=(kc == 0), stop=(kc == 7))
            for kc in range(8):
                xsq = TMP.tile([128, TOK], BF, tag="xsq")
                s.activation(xsq[:], xt[:, TOK * kc:TOK * (kc + 1)], Act.Square)
                te.matmul(sx2, onesb[:], xsq[:],
                          start=(kc == 0), stop=(kc == 7))
            # mu = sx/D ; var+eps = sx2/D - mu^2 + eps ; s1 = rsqrt
            v.tensor_scalar(rs("mu1"), sx, 1.0 / D, None, Alu.mult)
            v.tensor_tensor(rs("ra"), rs("mu1"), rs("mu1"), Alu.mult)
            v.tensor_scalar(rs("rb"), sx2, 1.0 / D, EPS, Alu.mult, Alu.add)
            v.tensor_tensor(rs("rb"), rs("rb"), rs("ra"), Alu.subtract)
            _quake_rsqrt(nc, rs("s1"), rs("rb"), rs("ra"), rs("rc"), rs("sc1"))
            mu_b = TMP.tile([128, TOK], F32, tag="mu_b", bufs=1)
            s_b = TMP.tile([128, TOK], F32, tag="s_b", bufs=1)
            g.partition_broadcast(mu_b[:], rs("mu1"))
            g.partition_broadcast(s_b[:], rs("s1"))
            tm = TMP.tile([128, TOK], F32, tag="tm")
            xlb = TMP.tile([128, 2048], BF, tag="xlb", bufs=1)
            for kc in range(8):
                v.tensor_tensor(tm[:], xt[:, TOK * kc:TOK * (kc + 1)],
                                mu_b[:], Alu.subtract)
                v.tensor_tensor(tm[:], tm[:], s_b[:], Alu.mult)
                v.tensor_scalar(xlb[:, TOK * kc:TOK * (kc + 1)], tm[:],
                                sm["lnw"][:, kc:kc + 1], sm["lnb"][:, kc:kc + 1],
                                Alu.mult, Alu.add)
            # ag_in[(kc*128+p), t] = xlb[p, kc*256+t]  (one strided DMA)
            dma(out=ag_in.ap().rearrange("(kc p) t -> p kc t", kc=8, p=128),
                in_=xlb[:].rearrange("p (kc t) -> p kc t", kc=8))
        g.collective_compute("AllGather", Alu.bypass, replica_groups=RG,
                             ins=[ag_in.ap()], outs=[ag_out.ap()])

        XA_cm = tc.tile_pool(name="xa_pool", bufs=1)
        XA = XA_cm.__enter__()
        xall = XA.tile([128, 16384], BF, tag="xall")
        # xall[p, kc*2048 + r*256 + t] = ag_out[(r*1024 + kc*128 + p), t]
        for kc in range(8):
            [nc.sync, nc.gpsimd][kc % 2].dma_start(
                out=xall[:, 2048 * kc:2048 * (kc + 1)]
                    .rearrange("p (r t) -> p r t", r=8),
                in_=ag_out.ap().rearrange("(r kc p) t -> kc p r t",
                                          r=8, kc=8, p=128)[kc])

        # resident prefetch: wout/wgram/wepc (consumed phases 5-6)
        wout_s = W3p.tile([128, 8192], BF, tag="wout_s")
        wgram_s = W3p.tile([128, 8192], BF, tag="wgram_s")
        wepc_s = W3p.tile([128, 2048], BF, tag="wepc_s")
        nc.scalar.dma_start(out=wout_s[:], in_=tin["wout"].ap())
        nc.gpsimd.dma_start(out=wgram_s[:], in_=tin["wgram"].ap())
        nc.scalar.dma_start(out=wepc_s[:], in_=tin["wepc"].ap())

        # ============ Phase 2: QKV + V(+ent) ============
        with tc.tile_pool(name="ps_qk", bufs=2, space="PSUM") as PSQ, \
             tc.tile_pool(name="ps_ev", bufs=2, space="PSUM") as PSV, \
             tc.tile_pool(name="esb", bufs=1) as ESB:
            for of in range(2):
                for w in range(4):
                    ps = PSQ.tile([128, 512], F32, tag="psqk")
                    for kc in range(8):
                        te.matmul(
                            ps[:],
                            wqk_s[:, (of * 8 + kc) * 128:(of * 8 + kc + 1) * 128],
                            xall[:, 2048 * kc + 512 * w:2048 * kc + 512 * (w + 1)],
                            start=(kc == 0), stop=(kc == 7))
                    v.tensor_scalar(
                        qkT[:, 2048 * of + 512 * w:2048 * of + 512 * (w + 1)],
                        ps[:], sm["b_qk"][:, of:of + 1], None, Alu.add)

            elog = ESB.tile([128, 16], F32, tag="elog")
            es = ESB.tile([128, 16], F32, tag="es")
            for tch in range(16):
                psv = PSV.tile([128, VW], F32, tag="psv")
                for kc in range(8):
                    te.matmul(
                        psv[:],
                        xall[:, 2048 * kc + 128 * tch:2048 * kc + 128 * (tch + 1)],
                        wv_s[:, VW * kc:VW * (kc + 1)],
                        start=(kc == 0), stop=(kc == 7))
                vt = vaug[:, VW * tch:VW * (tch + 1)]
                v.tensor_tensor(vt, psv[:], bvb[:], Alu.add)
                v.tensor_copy(elog[:, tch:tch + 1], psv[:, 136:137])
            # es = clip(sigmoid(elog + ent_b), 0.1, 2.0)
            s.activation(es[:], elog[:], Act.Exp,
                         bias=cst[:, 0:1], scale=-1.0)
            v.tensor_scalar(es[:], es[:], 1.0, None, Alu.add)
            v.reciprocal(es[:], es[:])
            v.tensor_scalar(es[:], es[:], 0.1, 2.0, Alu.max, Alu.min)
            for tch in range(16):
                vt = vaug[:, VW * tch:VW * tch + 136]
                v.tensor_scalar(vt, vt, es[:, tch:tch + 1], None, Alu.mult)
                for lh in range(HPC):
                    v.memset(vaug[:, VW * tch + 68 * lh + 64:
                                  VW * tch + 68 * lh + 65], 1.0)
            if dbg:
                dma(out=dbg["d_es"].ap()[:, 0:16], in_=es[:])

        if dbg:
            with tc.tile_pool(name="dbgx", bufs=1) as DBGX:
                for qq in range(8):
                    cvx = DBGX.tile([128, 2048], F32, tag="cvx")
                    v.tensor_copy(cvx[:], xall[:, 2048 * qq:2048 * (qq + 1)])
                    dma(out=dbg["d_xall"].ap()[:, 2048 * qq:2048 * (qq + 1)],
                        in_=cvx[:])
        XA_cm.__exit__(None, None, None)

        # ============ Phase 3: attention ============
        with tc.tile_pool(name="ps_sc", bufs=2, space="PSUM") as PSS, \
             tc.tile_pool(name="ps_ao", bufs=2, space="PSUM") as PSA, \
             tc.tile_pool(name="att_sb", bufs=3) as ASB, \
             tc.tile_pool(name="ao_sb", bufs=8) as AOSB, \
             tc.tile_pool(name="den_sb", bufs=2) as DSB:
            for lh in range(HPC):
                den8 = DSB.tile([128, 512], F32, tag="den8")
                att_stash = []
                hq = qkT[64 * lh:64 * (lh + 1), 0:2048]
                hk = qkT[64 * lh:64 * (lh + 1), 2048:4096]
                for G in range(4):
                    nkb = 4 * G + 4
                    ao = PSA.tile([65, 512], F32, tag="ao")
                    for pj in range(nkb // 2):
                        ps = PSS.tile([128, 1024], F32, tag="ps_sc")
                        ex = ASB.tile([128, 1024], BF, tag="ex")
                        for half in range(2):
                            kb = 2 * pj + half
                            te.matmul(ps[:, 512 * half:512 * (half + 1)],
                                      hk[:, 128 * kb:128 * (kb + 1)],
                                      hq[:, 512 * G:512 * (G + 1)],
                                      start=True, stop=True)
                        s.activation(ex[:], ps[:], Act.Exp)
                        for half in range(2):
                            kb = 2 * pj + half
                            j = kb - 4 * G
                            exh = ex[:, 512 * half:512 * (half + 1)]
                            if 0 <= j < 4:
                                v.tensor_tensor(
                                    exh, exh, tri_s[:, 512 * j:512 * (j + 1)],
                                    Alu.mult)
                            te.matmul(
                                ao[:],
                                vaug[:, VW * kb + 68 * lh:
                                     VW * kb + 68 * lh + 65],
                                exh,
                                start=(kb == 0), stop=(kb == nkb - 1))
                    aos = AOSB.tile([65, 512], F32, tag="aos")
                    s.copy(aos[:], ao[0:65, :])
                    v.tensor_copy(den8[32 * G:32 * G + 1, :], aos[64:65, :])
                    att_stash.append((G, aos))
                v.reciprocal(den8[:], den8[:])
                for G, aos in att_stash:
                    rrow = ASB.tile([1, 512], BF, tag="rrow")
                    v.tensor_copy(rrow[0:1, :], den8[32 * G:32 * G + 1, :])
                    rbp = PSA.tile([64, 512], F32, tag="rbp")
                    te.matmul(rbp[:], onesr[:], rrow[:], start=True, stop=True)
                    v.tensor_tensor(
                        aosc[64 * lh:64 * (lh + 1), 512 * G:512 * (G + 1)],
                        aos[0:64, :], rbp[:], Alu.mult)

        WA.__exit__(None, None, None)

        # ============ Phase 4: AllToAll ============
        dma(out=a2a_in.ap().rearrange("(r p) t -> p r t", r=8, p=128),
            in_=aosc[:].rearrange("p (r t) -> p r t", r=8))
        g.collective_compute("AllToAll", Alu.bypass, replica_groups=RG,
                             ins=[a2a_in.ap()], outs=[a2a_out.ap()])
        dma(out=aofull[:].rearrange("p (r t) -> p r t", r=8),
            in_=a2a_out.ap().rearrange("(r p) t -> p r t", r=8, p=128))

        # ============ Phase 5: out proj + norm1 ============
        with tc.tile_pool(name="ps_out", bufs=3, space="PSUM") as PSO, \
             tc.tile_pool(name="ps_r2", bufs=1, space="PSUM") as PSR2, \
             tc.tile_pool(name="tmp2", bufs=2) as TMP2:
            for of in range(8):
                ps = PSO.tile([128, TOK], F32, tag="ps_out")
                for kc in range(8):
                    te.matmul(
                        ps[:],
                        wout_s[:, (of * 8 + kc) * 128:(of * 8 + kc + 1) * 128],
                        aofull[:, TOK * kc:TOK * (kc + 1)],
                        start=(kc == 0), stop=(kc == 7))
                v.scalar_tensor_tensor(xt[:, TOK * of:TOK * (of + 1)],
                                       ps[:], sm["b_out"][:, of:of + 1],
                                       xt[:, TOK * of:TOK * (of + 1)],
                                       Alu.add, Alu.add)
            _ln_full(nc, tc, TMP2, PSR2, rs, xt, x1f, x1b, ones32,
                     sm["n1w"], sm["n1b"])

        MID_cm.__exit__(None, None, None)

        # ============ Phase 6: ep path + h-stats + ff1 ============
        with tc.tile_pool(name="ps_h", bufs=2, space="PSUM") as PSH, \
             tc.tile_pool(name="ps_r3", bufs=1, space="PSUM") as PSR3, \
             tc.tile_pool(name="tmp3b", bufs=1) as TMP3b:
            # --- mean of h from x1 (tiny) ---
            t_muh = PSR3.tile([1, 2 * TOK], F32, tag="muhp")
            pmu = t_muh[:, 0:TOK]
            psh2 = t_muh[:, TOK:2 * TOK]
            for kc in range(8):
                te.matmul(pmu, sm["wsum"][:, kc:kc + 1],
                          x1b[:, TOK * kc:TOK * (kc + 1)],
                          start=(kc == 0), stop=(kc == 7))
            v.tensor_scalar(rs("muh"), pmu, 1.0, sc["bsum"], Alu.mult, Alu.add)

            # --- sum of squares of h via Gram matrix ---
            for of in range(8):
                of2 = of % 2
                if of2 == 0:
                    ybf = TMP3b.tile([128, 2 * TOK], BF, tag="ybf", bufs=2)
                    zbf = TMP3b.tile([128, 2 * TOK], BF, tag="zbf", bufs=2)
                ps = PSH.tile([128, TOK], F32, tag="ps_h")
                for kc in range(8):
                    te.matmul(
                        ps[:],
                        wgram_s[:, (of * 8 + kc) * 128:(of * 8 + kc + 1) * 128],
                        x1b[:, TOK * kc:TOK * (kc + 1)],
                        start=(kc == 0), stop=(kc == 7))
                s.activation(ybf[:, TOK * of2:TOK * (of2 + 1)], ps[:],
                             Act.Identity, bias=sm["c_lin"][:, of:of + 1])
                v.tensor_tensor(zbf[:, TOK * of2:TOK * (of2 + 1)],
                                ybf[:, TOK * of2:TOK * (of2 + 1)],
                                x1b[:, TOK * of:TOK * (of + 1)], Alu.mult)
                te.matmul(psh2, onesb[:], zbf[:, TOK * of2:TOK * (of2 + 1)],
                          start=(of == 0), stop=(of == 7))
            # var+eps = sh2/FF + btb/FF + eps - muh^2 ; S = rsqrt(.)/65
            v.tensor_tensor(rs("ra"), rs("muh"), rs("muh"), Alu.mult)
            v.tensor_scalar(rs("rb"), psh2, 1.0 / FF,
                            sc["btb"] / FF + EPS, Alu.mult, Alu.add)
            v.tensor_tensor(rs("rb"), rs("rb"), rs("ra"), Alu.subtract)
            _quake_rsqrt(nc, rs("Sh"), rs("rb"), rs("ra"), rs("rc"), rs("sc1"),
                         scale=1.0 / (1.0 + np.sqrt(FF)))
            v.tensor_tensor(rs("muS"), rs("muh"), rs("Sh"), Alu.mult)

            # --- ep gate path (contracted over D) ---
            t_se1 = PSR3.tile([1, TOK], F32, tag="se1p")
            t_se2 = PSR3.tile([1, TOK], F32, tag="se2p")
            se1 = t_se1[:]
            se2 = t_se2[:]
            t_pse2 = PSR3.tile([1, TOK], F32, tag="pse2p")
            pse2 = t_pse2[:]
            epb = TMP3b.tile([128, 2 * TOK], BF, tag="epb")
            epsq = TMP3b.tile([128, TOK], BF, tag="epsq")
            for of in range(2):
                ps = PSH.tile([128, TOK], F32, tag="ps_h")
                for kc in range(8):
                    te.matmul(
                        ps[:],
                        wepc_s[:, (of * 8 + kc) * 128:(of * 8 + kc + 1) * 128],
                        x1b[:, TOK * kc:TOK * (kc + 1)],
                        start=(kc == 0), stop=(kc == 7))
                s.activation(epb[:, TOK * of:TOK * (of + 1)], ps[:],
                             Act.Identity, bias=sm["b_epc"][:, of:of + 1])
                s.activation(epsq[:], ps[:], Act.Square,
                             bias=sm["b_epc"][:, of:of + 1])
                te.matmul(se1, onesb[:], epb[:, TOK * of:TOK * (of + 1)],
                          start=(of == 0), stop=(of == 1))
                te.matmul(se2, onesb[:], epsq[:],
                          start=(of == 0), stop=(of == 1))
            v.tensor_scalar(rs("mue"), se1, 1.0 / D16, None, Alu.mult)
            v.tensor_tensor(rs("ra"), rs("mue"), rs("mue"), Alu.mult)
            v.tensor_scalar(rs("rb"), se2, 1.0 / D16, EPS, Alu.mult, Alu.add)
            v.tensor_tensor(rs("rb"), rs("rb"), rs("ra"), Alu.subtract)
            _quake_rsqrt(nc, rs("se"), rs("rb"), rs("ra"), rs("rc"), rs("sc1"))
            mue_b = TMP3b.tile([128, TOK], F32, tag="mue_b")
            see_b = TMP3b.tile([128, TOK], F32, tag="see_b")
            g.partition_broadcast(mue_b[:], rs("mue"))
            g.partition_broadcast(see_b[:], rs("se"))
            relub = TMP3b.tile([128, 2 * TOK], BF, tag="relub")
            tm3 = TMP3b.tile([128, TOK], F32, tag="tm3")
            for of in range(2):
                v.tensor_tensor(tm3[:], epb[:, TOK * of:TOK * (of + 1)],
                                mue_b[:], Alu.subtract)
                v.tensor_tensor(tm3[:], tm3[:], see_b[:], Alu.mult)
                s.activation(relub[:, TOK * of:TOK * (of + 1)], tm3[:],
                             Act.Relu, bias=sm["eplb"][:, of:of + 1],
                             scale=sm["eplw"][:, of:of + 1])
            for of in range(2):
                te.matmul(pse2, sm["wep2"][:, of:of + 1],
                          relub[:, TOK * of:TOK * (of + 1)],
                          start=(of == 0), stop=(of == 1))
            # em = 1 + 0.1*sigmoid(pse2 + ep2_b)
            s.activation(rs("em"), pse2, Act.Exp, bias=cst[0:1, 1:2], scale=-1.0)
            v.tensor_scalar(rs("em"), rs("em"), 1.0, None, Alu.add)
            v.reciprocal(rs("em"), rs("em"))
            v.tensor_scalar(rs("em"), rs("em"), 0.1, 1.0, Alu.mult, Alu.add)

            # --- ff1 (weights streamed per chunk) ---
            hb = HBp.tile([128, 8192], BF, tag="hb")
            for c in range(32):
                w1t = WF1p.tile([128, 1024], BF, tag="w1t")
                [nc.sync, nc.gpsimd][c % 2].dma_start(
                    out=w1t[:], in_=tin["wff1"].ap()[:, 1024 * c:1024 * (c + 1)])
                ps = PSH.tile([128, TOK], F32, tag="ps_h")
                for kc in range(8):
                    te.matmul(ps[:],
                              w1t[:, 128 * kc:128 * (kc + 1)],
                              x1b[:, TOK * kc:TOK * (kc + 1)],
                              start=(kc == 0), stop=(kc == 7))
                s.activation(hb[:, TOK * c:TOK * (c + 1)], ps[:], Act.Identity,
                             bias=sm["b_ff1"][:, c:c + 1])

            # broadcast per-token spline rows
            Sh_b = TMP3.tile([128, TOK], F32, tag="Sh_b")
            muS_b = TMP3.tile([128, TOK], F32, tag="muS_b")
            em_b = TMP3.tile([128, TOK], F32, tag="em_b")
            g.partition_broadcast(Sh_b[:], rs("Sh"))
            g.partition_broadcast(muS_b[:], rs("muS"))
            g.partition_broadcast(em_b[:], rs("em"))
            Srep = TMP3.tile([128, 2048], BF, tag="Srep")
            muSrep = TMP3.tile([128, 2048], BF, tag="muSrep")
            emrep = TMP3.tile([128, 2048], BF, tag="emrep")
            for src8, t8 in ((Sh_b, Srep), (muS_b, muSrep), (em_b, emrep)):
                v.tensor_copy(t8[:], src8[:].unsqueeze(1)
                              .to_broadcast((128, 8, TOK)))
        # ============ Phase 7: spline + ff2 interleaved ============
        if True:
            a_q, d_q, s2, c0p, c3 = (spl["a"], spl["d"], spl["s2"],
                                     spl["c0p"], spl["c3"])
            op_q = Alu.add if s2 > 0 else Alu.subtract
            with tc.tile_pool(name="spl_sb", bufs=2) as SPL:
              r2 = SPL.tile([128, 8 * TOK], F32, tag="r2", bufs=1)
              with tc.tile_pool(name="ps_f2", bufs=1, space="PSUM") as PSF:
                psF = [PSF.tile([128, TOK], F32, tag=f"psF{i}", name=f"psF{i}")
                       for i in range(8)]
                DBGU_cm = tc.tile_pool(name="dbgu", bufs=1) if dbg else None
                DBGU = DBGU_cm.__enter__() if dbg else None
                for gi in range(4):
                    w2t = WF2p.tile([128, 8192], BF, tag="w2t")
                    [nc.sync, nc.gpsimd][gi % 2].dma_start(
                        out=w2t[:],
                        in_=tin["wff2"].ap()[:, 8192 * gi:8192 * (gi + 1)])
                    hbs = hb[:, 2048 * gi:2048 * (gi + 1)]
                    u = SPL.tile([128, 2048], BF, tag="u")
                    q = SPL.tile([128, 2048], BF, tag="q")
                    t3 = SPL.tile([128, 2048], BF, tag="t3")
                    acc = SPL.tile([128, 2048], BF, tag="acc")
                    actt = SPL.tile([128, 2048], BF, tag="actt")
                    v.tensor_tensor(u[:], hbs, Srep[:], Alu.mult)
                    v.tensor_tensor(u[:], u[:], muSrep[:], Alu.subtract)
                    s.activation(q[:], u[:], Act.Square, bias=cst[:, 2:3], scale=a_q)
                    s.activation(t3[:], u[:], Act.Abs)
                    v.scalar_tensor_tensor(acc[:], t3[:], c3, q[:],
                                           Alu.mult, op_q)
                    v.tensor_scalar(acc[:], acc[:], c0p, None, Alu.add)
                    v.tensor_tensor(acc[:], acc[:], emrep[:], Alu.mult)
                    v.tensor_scalar(actt[:], acc[:], 1.0, -1.0,
                                    Alu.min, Alu.max)
                    for kc8 in range(8):
                        kc = 8 * gi + kc8
                        for of in range(8):
                            te.matmul(
                                psF[of][:],
                                w2t[:, (kc8 * 8 + of) * 128:(kc8 * 8 + of + 1) * 128],
                                actt[:, TOK * kc8:TOK * (kc8 + 1)],
                                start=(kc == 0), stop=(kc == 31))
                    if dbg:
                        cvu = DBGU.tile([128, 2048], F32, tag="cvu")
                        v.tensor_copy(cvu[:], u[:])
                        dma(out=dbg["d_u"].ap()[:, 2048 * gi:2048 * (gi + 1)],
                            in_=cvu[:])
                        cva = DBGU.tile([128, 2048], F32, tag="cva")
                        v.tensor_copy(cva[:], actt[:])
                        dma(out=dbg["d_actt"].ap()[:, 2048 * gi:2048 * (gi + 1)],
                            in_=cva[:])

                if dbg:
                    DBGU_cm.__exit__(None, None, None)
                # ff2 epilogue (inside PSF scope)
                for of in range(8):
                    v.scalar_tensor_tensor(
                        r2[:, TOK * of:TOK * (of + 1)],
                        psF[of][:],
                        sm["b_ff2"][:, of:of + 1],
                        x1f[:, TOK * of:TOK * (of + 1)],
                        Alu.add, Alu.add)
              # ============ Phase 8: norm2 ============
              with tc.tile_pool(name="ps_r4", bufs=1, space="PSUM") as PSR4, \
                   tc.tile_pool(name="tmp4", bufs=2) as TMP4:
                    t_sx4 = PSR4.tile([1, 2 * TOK], F32, tag="lnsx4")
                    sx4 = t_sx4[:, 0:TOK]
                    sx42 = t_sx4[:, TOK:2 * TOK]
                    for of in range(8):
                        te.matmul(sx4, ones32[:], r2[:, TOK * of:TOK * (of + 1)],
                                  start=(of == 0), stop=(of == 7))
                    xsq4 = TMP4.tile([128, TOK], F32, tag="xsq4")
                    for of in range(8):
                        s.activation(xsq4[:], r2[:, TOK * of:TOK * (of + 1)],
                                     Act.Square)
                        te.matmul(sx42, ones32[:], xsq4[:],
                                  start=(of == 0), stop=(of == 7))
                    if dbg:
                        dma(out=dbg["d_r2"].ap(), in_=r2[:])
                    v.tensor_scalar(rs("m1"), sx4, 1.0 / D, None, Alu.mult)
                    v.tensor_tensor(rs("ra"), rs("m1"), rs("m1"), Alu.mult)
                    v.tensor_scalar(rs("rb"), sx42, 1.0 / D, EPS,
                                    Alu.mult, Alu.add)
                    v.tensor_tensor(rs("rb"), rs("rb"), rs("ra"), Alu.subtract)
                    _quake_rsqrt(nc, rs("m2"), rs("rb"), rs("ra"), rs("rc"),
                                 rs("sc1"))
                    mu4_b = TMP4.tile([128, TOK], F32, tag="mu4_b", bufs=1)
                    s4_b = TMP4.tile([128, TOK], F32, tag="s4_b", bufs=1)
                    g.partition_broadcast(mu4_b[:], rs("m1"))
                    g.partition_broadcast(s4_b[:], rs("m2"))
                    tm4 = TMP4.tile([128, TOK], F32, tag="tm4")
                    for of in range(8):
                        v.tensor_tensor(tm4[:], r2[:, TOK * of:TOK * (of + 1)],
                                        mu4_b[:], Alu.subtract)
                        v.tensor_tensor(tm4[:], tm4[:], s4_b[:], Alu.mult)
                        v.tensor_scalar(xt[:, TOK * of:TOK * (of + 1)], tm4[:],
                                        sm["n2w"][:, of:of + 1],
                                        sm["n2b"][:, of:of + 1],
                                        Alu.mult, Alu.add)
                        if of % 4 == 3:
                            dma(out=t_out.ap()[:, TOK * (of - 3):TOK * (of + 1)],
                                in_=xt[:, TOK * (of - 3):TOK * (of + 1)])
            HB_cm.__exit__(None, None, None)
            WF2.__exit__(None, None, None)
            WF1.__exit__(None, None, None)
            W3.__exit__(None, None, None)

        TMP3_cm.__exit__(None, None, None)
        # (HB/MID closed above)
        if dbg:
            with tc.tile_pool(name="dbgp", bufs=1) as DBG:
                def dump(name, tile_ap, width):
                    nch = max(1, width // 2048)
                    w = width // nch
                    for qq in range(nch):
                        cv = DBG.tile([128, w], F32, tag="cv",
                                      name=f"cv{name}{qq}")
                        v.tensor_copy(cv[:], tile_ap[:, w * qq:w * (qq + 1)])
                        dma(out=dbg[name].ap()[:, w * qq:w * (qq + 1)],
                            in_=cv[:])
                dump("d_qkT", qkT[:], 4096)
                dump("d_vaug", vaug[:], 16 * VW)
                dump("d_aofull", aofull[:], 8 * TOK)
                dma(out=dbg["d_x1f"].ap(), in_=x1f[:])
                dma(out=dbg["d_rows"].ap()[:, 0:NROW * TOK], in_=rows[:])


def _ln_full(nc, tc, TMP, PSR, rs, src, dstf, dstb, ones32, wcol, bcol):
    v, s, g, te = nc.vector, nc.scalar, nc.gpsimd, nc.tensor
    T = TOK
    t_sx = PSR.tile([1, 2 * T], F32, tag="lnsxp")
    sx = t_sx[:, 0:T]
    sx2 = t_sx[:, T:2 * T]
    for kc in range(8):
        te.matmul(sx, ones32[:], src[:, T * kc:T * (kc + 1)],
                  start=(kc == 0), stop=(kc == 7))
    xsq = TMP.tile([128, T], F32, tag="lnxsq")
    for kc in range(8):
        s.activation(xsq[:], src[:, T * kc:T * (kc + 1)], Act.Square)
        te.matmul(sx2, ones32[:], xsq[:], start=(kc == 0), stop=(kc == 7))
    v.tensor_scalar(rs("m1"), sx, 1.0 / D, None, Alu.mult)
    v.tensor_tensor(rs("ra"), rs("m1"), rs("m1"), Alu.mult)
    v.tensor_scalar(rs("rb"), sx2, 1.0 / D, EPS, Alu.mult, Alu.add)
    v.tensor_tensor(rs("rb"), rs("rb"), rs("ra"), Alu.subtract)
    _quake_rsqrt(nc, rs("m2"), rs("rb"), rs("ra"), rs("rc"), rs("sc1"))
    mu_b = TMP.tile([128, T], F32, tag="lnmu_b")
    s_b = TMP.tile([128, T], F32, tag="lns_b")
    g.partition_broadcast(mu_b[:], rs("m1"))
    g.partition_broadcast(s_b[:], rs("m2"))
    tm = TMP.tile([128, T], F32, tag="lntm")
    for kc in range(8):
        v.tensor_tensor(tm[:], src[:, T * kc:T * (kc + 1)], mu_b[:],
                        Alu.subtract)
        v.tensor_tensor(tm[:], tm[:], s_b[:], Alu.mult)
        v.tensor_scalar(dstf[:, T * kc:T * (kc + 1)], tm[:],
                        wcol[:, kc:kc + 1], bcol[:, kc:kc + 1],
                        Alu.mult, Alu.add)
        if dstb is not None:
            s.activation(dstb[:, T * kc:T * (kc + 1)], tm[:], Act.Identity,
                         bias=bcol[:, kc:kc + 1], scale=wcol[:, kc:kc + 1])


# ----------------------------------------------------------------------------
# Entry point
# ----------------------------------------------------------------------------

def kernel(**inputs):
    in_maps, sc = _prepare_inputs(inputs)
    key = hashlib.sha256(
        repr((sc["ent_b"], sc["ep2_b"], sc["bsum"], sc["btb"],
              sorted(sc["spl"].items()))).encode()
    ).hexdigest()
    if key not in _prog_cache:
        _prog_cache[key] = _build_program(sc)
    nc = _prog_cache[key]
    res = bass_utils.run_bass_kernel_spmd(nc, in_maps,
                                          core_ids=list(range(NCORES)))
    out = np.empty((1, S, D), np.float32)
    for c in range(NCORES):
        oc = np.asarray(res.results[c]["out"], np.float32)   # [128, 8*TOK]
        ot = oc.reshape(128, 8, TOK).transpose(1, 0, 2).reshape(D, TOK)
        out[0, c * TOK:(c + 1) * TOK, :] = ot.T
    return out
